# revision 1
# baseline (speedup 1.0000x reference)
"""3-layer GCN + mean-pool + classifier for Trainium2, SPMD on 8 NeuronCores.

Self-contained: kernel(**inputs) takes the full-size numpy inputs, does the
host-side graph partitioning, builds/compiles a Bass/Tile kernel, runs it on
cores 0-7 via run_bass_kernel_spmd, and returns the [128, 3] log-softmax
output.

Distribution: nodes are dst-sharded across the 8 cores. Per GCN layer each
core computes t' = dinv * (h @ W) for its shard (TensorE), the shards are
AllGathered into a full node-major table, each core dma_gathers its in-edges'
source rows (256B rows) and dma_scatter_adds them into its local accumulator
table. Scatter calls are organized into rounds of unique destination rows
(the SDMA CCE read-modify-write loses updates when one call carries duplicate
indices); nodes with degree > LV overflow into chained virtual rows that are
merged back hierarchically. HW limits found empirically: gather index values
must be < 8192 (16 source buckets), gather calls <= 1024 indices, scatter
calls <= 2048. The GCN normalization deg^-1/2 (A+I) deg^-1/2 factorizes into
a pre-scale of t' and a post-scale of the aggregate, so no per-edge weight is
needed; self-loops become a prefill of the accumulator with t'. Mean-pooling
runs as a one-hot matmul on TensorE with an AllReduce of per-core partials;
the classifier + log_softmax run replicated on every core.
"""
import sys

sys.path.insert(0, "/opt/trn_rl_repo")

import numpy as np
import concourse.bacc as bacc
import concourse.mybir as mybir
import concourse.tile as tile
from concourse.masks import make_identity
import concourse.tile as _tile
import concourse.mybir as _mybir
from concourse.vector_clock import ScopedClock as _ScopedClock

# ---------------------------------------------------------------------------
# Workarounds: this walrus build rejects >1 sync-wait per instruction.

import concourse.tile as _tile
import concourse.mybir as _mybir
from concourse.vector_clock import ScopedClock as _ScopedClock


def _split_waits_tail(nc, inst):
    si = inst.ins.sync_info
    if si is None or not si.on_wait or len(si.on_wait) <= 1:
        return
    waits = list(si.on_wait)
    inst.ins.sync_info = _mybir.SyncInfo(on_wait=[], on_update=list(si.on_update or []))
    for w in waits:
        nop = nc.sync.nop()
        nop.ins.sync_info = _mybir.SyncInfo(on_wait=[w], on_update=[])


def _drain_and_barrier(self, tick_clock, wait_clock):
    nc = self.nc
    probe = nc.sync.nop()
    wait_clock.add_sem_waits(probe.ins, _ScopedClock({None: tick_clock.global_clock}))
    _split_waits_tail(nc, probe)
    nc.sync.drain()
    nc.all_engine_barrier()
    assert self.sems is not None
    popped = nc._tile_sem_poison_stack.pop()
    assert popped is self._sem_poison
    nc.clear_and_free_semaphores(list(self.sems.allocated().values()))
    nc.all_engine_barrier()


_tile.TileContext._drain_and_barrier = _drain_and_barrier


def fix_multiwait(nc):
    """Rewrite every >1-wait instruction into wait-nops + 1-wait instruction."""
    for f in nc.m.functions:
        for blk in f.blocks:
            insts = blk.instructions            # live list (rust-backed)
            i = 0
            while i < len(insts):
                inst = insts[i]
                si = inst.sync_info
                if si is not None and si.on_wait and len(si.on_wait) > 1:
                    waits = list(si.on_wait)
                    eng = inst.engine
                    inst.sync_info = _mybir.SyncInfo(
                        on_wait=[waits[-1]], on_update=list(si.on_update or [])
                    )
                    for j, w in enumerate(waits[:-1]):
                        nop = nc.engines[eng].nop(hint="mwfix")
                        popped = False
                        for f2 in nc.m.functions:
                            for b2 in f2.blocks:
                                l2 = b2.instructions
                                if l2 and l2[-1].name == nop.ins.name:
                                    l2.pop()
                                    popped = True
                                    break
                            if popped:
                                break
                        assert popped, "could not relocate mwfix nop"
                        nop.ins.sync_info = _mybir.SyncInfo(on_wait=[w], on_update=[])
                        insts.insert(i + j, nop.ins)
                    i += len(waits) - 1
                i += 1


# ---------------------------------------------------------------------------

import numpy as np
import concourse.bacc as bacc
import concourse.mybir as mybir
import concourse.tile as tile
from concourse.masks import make_identity

F32 = mybir.dt.float32
I16 = mybir.dt.int16
AF = mybir.ActivationFunctionType
ALU = mybir.AluOpType


def cdiv(a, b):
    return (a + b - 1) // b


def rup(a, b):
    return cdiv(a, b) * b


class Cfg:
    def __init__(self, N, E, IN, HID, G, OUT, LV=20):
        self.C = 8
        self.N, self.E, self.IN, self.HID, self.G, self.OUT = N, E, IN, HID, G, OUT
        assert N % self.C == 0
        self.NSH = N // self.C
        self.TROW = rup(self.NSH, 128)
        self.NCHK = self.TROW // 128
        self.NBUK = 16
        assert (self.C * self.TROW) % self.NBUK == 0
        self.SRCW = self.C * self.TROW // self.NBUK
        assert self.SRCW <= 8191  # HW: gather idx value must fit 13 bits
        self.LV = LV                      # rounds per row (round cap)
        assert G <= 128


def _ranks(dst):
    """rank of each element within its dst group (stable)."""
    n = len(dst)
    order = np.lexsort((np.arange(n), dst))
    sd = dst[order]
    first = np.r_[0, np.flatnonzero(np.diff(sd)) + 1]
    sizes = np.diff(np.r_[first, n])
    grp_start = np.repeat(first, sizes)
    rank_sorted = np.arange(n) - grp_start
    rank = np.empty(n, np.int64)
    rank[order] = rank_sorted
    return rank


def _wrap_cols(a):
    """[n sl ots] (n % 16 == 0) -> wrapped [128, n // 16] int16."""
    w = a.reshape(-1, 16).T  # [16, n//16]
    return np.tile(w, (8, 1)).astype(np.int16)


def prep(inputs, cfg):
    c = cfg
    x = np.asarray(inputs["x"], np.float32)
    ei = np.asarray(inputs["edge_index"], np.int64)
    batch = np.asarray(inputs["batch"], np.int64)
    W1 = np.asarray(inputs["W1"], np.float32); b1 = np.asarray(inputs["b1"], np.float32)
    W2 = np.asarray(inputs["W2"], np.float32); b2 = np.asarray(inputs["b2"], np.float32)
    W3 = np.asarray(inputs["W3"], np.float32); b3 = np.asarray(inputs["b3"], np.float32)
    Wc = np.asarray(inputs["Wc"], np.float32); bc = np.asarray(inputs["bc"], np.float32)

    src = ei[0].astype(np.int64)
    dst = ei[1].astype(np.int64)
    deg = np.bincount(dst, minlength=c.N).astype(np.float32) + 1.0
    dinv = 1.0 / np.sqrt(deg)

    HID = c.HID
    W3p = np.zeros((HID, HID), np.float32); W3p[:, : W3.shape[1]] = W3
    b3p = np.zeros((HID,), np.float32); b3p[: b3.shape[0]] = b3
    Wcp = np.zeros((HID, c.OUT), np.float32); Wcp[: Wc.shape[0]] = Wc

    core_of = src // c.NSH
    trow_src = core_of * c.TROW + (src - core_of * c.NSH)
    buk_all = trow_src // c.SRCW
    gidx_all = trow_src - buk_all * c.SRCW
    dcore = dst // c.NSH

    LV = c.LV
    DUMP = c.TROW            # dump row (pads), rows TROW..TROW+127 unused
    VBASE = c.TROW + 128     # virtual rows start here

    percore = []
    maxdeg = 0
    for ci in range(c.C):
        m = dcore == ci
        e_g = gidx_all[m]
        e_b = buk_all[m]
        d_loc = dst[m] - ci * c.NSH
        rank = _ranks(d_loc)
        maxdeg = max(maxdeg, int(rank.max(initial=0)) + 1)
        percore.append(dict(e_g=e_g, e_b=e_b, d_loc=d_loc, rank=rank,
                            lvl=rank // LV))
    NLVL = cdiv(maxdeg, LV) - 1          # number of virtual levels (>=0)
    nvr_max = [0] * NLVL                 # cross-core max vrows per level
    for pc in percore:
        for L in range(1, NLVL + 1):
            nvr_max[L - 1] = max(nvr_max[L - 1],
                                 len(np.unique(pc["d_loc"][pc["lvl"] >= L])))
    NV = [rup(max(n, 1), 128) for n in nvr_max]
    VLBASE = []
    base = VBASE
    for L in range(NLVL):
        VLBASE.append(base)
        base += NV[L]
    AGGROWS = base

    # per-core final rows + per-(core,round,bucket) counts
    R = min(LV, maxdeg)
    cnts = np.zeros((c.C, R, c.NBUK), np.int64)
    for ci in range(c.C):
        pc = percore[ci]
        vmaps = []
        for L in range(1, NLVL + 1):
            uds = np.unique(pc["d_loc"][pc["lvl"] >= L])
            vmaps.append({d: VLBASE[L - 1] + i for i, d in enumerate(uds)})
        frow = pc["d_loc"].copy()
        l = pc["lvl"]
        for L in range(1, NLVL + 1):
            if (l == L).any():
                frow[l == L] = np.array(
                    [vmaps[L - 1][d] for d in pc["d_loc"][l == L]], np.int64)
        pc["frow"] = frow
        pc["frank"] = pc["rank"] % LV
        pc["vmaps"] = vmaps
        for r in range(R):
            sel = pc["frank"] == r
            for b in range(c.NBUK):
                cnts[ci, r, b] = int((sel & (pc["e_b"] == b)).sum())
    SEG = np.zeros((R, c.NBUK), np.int64)
    for r in range(R):
        for b in range(c.NBUK):
            mx = int(cnts[:, r, b].max())
            SEG[r, b] = rup(mx, 128) if mx > 0 else 0
    # pack each round's bucket segments into bins of <= MAXMSG slots; one
    # scatter call per bin (unique rows within a round => within a bin).
    # HW limit: gather calls take at most MAXG indices, so split big segments.
    MAXMSG = 2048
    MAXG = 1024
    CALLS = []   # (bin_slots, scol, [(bucket, seg, col), ...])
    col = 0
    for r in range(R):
        cur = []
        cur_sz = 0
        scol = col
        for b in range(c.NBUK):
            seg = int(SEG[r, b])
            if seg == 0:
                continue
            while seg > 0:
                sub = min(seg, MAXG)
                if cur_sz + sub > MAXMSG:
                    CALLS.append((cur_sz, scol, cur))
                    cur, cur_sz, scol = [], 0, col
                cur.append((b, sub, col))
                cur_sz += sub
                col += sub
                seg -= sub
        if cur:
            CALLS.append((cur_sz, scol, cur))
    TOT = col
    RSZ = SEG.sum(axis=1)

    # merge calls, deepest level first: L -> L-1 -> ... -> real rows
    VM = [NV[L] for L in reversed(range(NLVL))]
    MTOT = sum(VM)
    # split each level's merge into <= MAXG sub-calls (disjoint rows => safe)
    MCALLS = []
    for v in VM:
        while v > 0:
            sub = min(v, MAXG)
            MCALLS.append(sub)
            v -= sub

    GCOLS = (TOT + MTOT) // 16
    SCOLS = (TOT + MTOT) // 16

    cnt = np.bincount(batch, minlength=c.G).astype(np.float32)
    cntinv = (1.0 / np.maximum(cnt, 1.0)).astype(np.float32)

    in_maps = []
    for ci in range(c.C):
        pc = percore[ci]
        lo, hi = ci * c.NSH, (ci + 1) * c.NSH
        xT = np.zeros((c.IN, c.TROW), np.float32)
        xT[:, : c.NSH] = x[lo:hi].T
        dv = np.zeros((c.TROW,), np.float32)
        dv[: c.NSH] = dinv[lo:hi]
        dinv2d = dv.reshape(c.NCHK, 128).T.copy()

        g_slots = np.zeros(TOT + MTOT, np.int64)           # gather idx per slot
        s_slots = np.full(TOT + MTOT, DUMP, np.int64)      # scatter idx per slot
        off = 0
        for r in range(R):
            selr = pc["frank"] == r
            for b in range(c.NBUK):
                sel = selr & (pc["e_b"] == b)
                k = int(sel.sum())
                g_slots[off: off + k] = pc["e_g"][sel]
                s_slots[off: off + k] = pc["frow"][sel]
                # sanity: unique dst within the round call
                off += int(SEG[r, b])
        assert off == TOT
        # merge slots: gather from aggb vrows, scatter to parents (deepest 1st)
        vmaps = pc["vmaps"]
        for mi, L in enumerate(reversed(range(1, NLVL + 1))):
            items = sorted(vmaps[L - 1].items(), key=lambda kv: kv[1])
            for i, (d, vr) in enumerate(items):
                g_slots[off + i] = vr
                s_slots[off + i] = vmaps[L - 2][d] if L >= 2 else d
            g_slots[off + len(items): off + VM[mi]] = DUMP
            off += VM[mi]
        assert off == TOT + MTOT

        # uniqueness check per scatter call (excluding DUMP pads)
        for bin_sz, scol, _ in CALLS:
            ss = s_slots[scol: scol + bin_sz]
            real = ss[ss != DUMP]
            assert len(np.unique(real)) == len(real), "dup within scatter call"

        gidx_w = _wrap_cols(g_slots)                       # [128, GCOLS]
        sidx_w = _wrap_cols(s_slots)

        oneh = np.zeros((c.TROW, 128), np.float32)
        oneh[np.arange(c.NSH), batch[lo:hi].astype(np.int64)] = 1.0

        bcols = np.stack([b1, b2, b3p], axis=1)
        b3rep = np.tile(b3p[None, :], (128, 1))
        bcrep = np.tile(bc[None, :], (128, 1))
        cinv = np.zeros((128, 1), np.float32)
        cinv[: c.G, 0] = cntinv

        in_maps.append(dict(
            xT=xT, dinv2d=dinv2d, gidx=gidx_w, sidx=sidx_w, oneh=oneh,
            W1d=W1, W2d=W2, W3d=W3p, bcols=bcols, b3rep=b3rep,
            Wcp=Wcp, bcrep=bcrep, cinv=cinv,
        ))

    meta = dict(R=R, CALLS=CALLS, TOT=TOT, MAXMSG=MAXMSG,
                VM=MCALLS, GCOLS=GCOLS, AGGROWS=AGGROWS, DUMP=DUMP)
    return in_maps, meta


def build(cfg, meta):
    c = cfg
    HID, G, OUT = c.HID, c.G, c.OUT
    CALLS, TOT, VM = meta["CALLS"], meta["TOT"], meta["VM"]
    GCOLS = meta["GCOLS"]
    AGGROWS = meta["AGGROWS"]
    MAXMSG = meta["MAXMSG"]
    assert max(VM) <= MAXMSG

    nc = bacc.Bacc("TRN2", num_devices=c.C, dynamic_dma_scratch_size=65536)

    def ein(name, shape, dt=F32):
        return nc.dram_tensor(name, shape, dt, kind="ExternalInput")

    xT_d = ein("xT", [c.IN, c.TROW])
    dinv_d = ein("dinv2d", [128, c.NCHK])
    gidx_d = ein("gidx", [128, GCOLS], I16)
    sidx_d = ein("sidx", [128, GCOLS], I16)
    oneh_d = ein("oneh", [c.TROW, 128])
    W1_d = ein("W1d", [c.IN, HID])
    W2_d = ein("W2d", [HID, HID])
    W3_d = ein("W3d", [HID, HID])
    bcols_d = ein("bcols", [HID, 3])
    b3rep_d = ein("b3rep", [128, HID])
    Wc_d = ein("Wcp", [HID, OUT])
    bcrep_d = ein("bcrep", [128, OUT])
    cinv_d = ein("cinv", [128, 1])

    agin_d = nc.dram_tensor("agin", [c.TROW, HID], F32, kind="Internal")
    agout_d = nc.dram_tensor(
        "agout", [c.C * c.TROW, HID], F32, kind="Internal", addr_space="Shared")
    aggb_d = nc.dram_tensor("aggb", [AGGROWS, HID], F32, kind="Internal")
    plin_d = nc.dram_tensor("plin", [128, HID], F32, kind="Internal")
    plout_d = nc.dram_tensor(
        "plout", [128, HID], F32, kind="Internal", addr_space="Shared")
    y_d = nc.dram_tensor("y", [G, OUT], F32, kind="ExternalOutput")

    rg = [list(range(c.C))]
    NVTOT = AGGROWS - c.TROW            # dump + virtual rows region

    with tile.TileContext(nc) as tc:
        with (
            tc.tile_pool(name="res", bufs=1) as res,
            tc.tile_pool(name="stage", bufs=1) as stpool,
            tc.tile_pool(name="work", bufs=6) as work,
            tc.tile_pool(name="msgs", bufs=6) as msgs,
            tc.tile_pool(name="psA", bufs=2, space="PSUM") as psA,
            tc.tile_pool(name="psB", bufs=2, space="PSUM") as psB,
        ):
            ident = res.tile([128, 128], F32)
            make_identity(nc, ident[:])
            dinv_sb = res.tile([128, c.NCHK], F32)
            nc.sync.dma_start(dinv_sb[:], dinv_d[:])
            W_sb = [res.tile([c.IN, HID], F32, name="w1"),
                    res.tile([HID, HID], F32, name="w2"),
                    res.tile([HID, HID], F32, name="w3")]
            nc.sync.dma_start(W_sb[0][:], W1_d[:])
            nc.sync.dma_start(W_sb[1][:], W2_d[:])
            nc.sync.dma_start(W_sb[2][:], W3_d[:])
            bcols_sb = res.tile([HID, 3], F32)
            nc.sync.dma_start(bcols_sb[:], bcols_d[:])
            b3rep_sb = res.tile([128, HID], F32)
            nc.sync.dma_start(b3rep_sb[:], b3rep_d[:])
            Wc_sb = res.tile([HID, OUT], F32)
            nc.sync.dma_start(Wc_sb[:], Wc_d[:])
            bcrep_sb = res.tile([128, OUT], F32)
            nc.sync.dma_start(bcrep_sb[:], bcrep_d[:])
            cinv_sb = res.tile([128, 1], F32)
            nc.sync.dma_start(cinv_sb[:], cinv_d[:])
            zero_sb = res.tile([128, cdiv(NVTOT, 128), HID], F32)
            nc.vector.memset(zero_sb[:], 0.0)

            hT_sb = stpool.tile([HID, c.TROW], F32)
            stage_sb = stpool.tile([128, c.NCHK, HID], F32)
            h3_sb = stpool.tile([128, c.NCHK, HID], F32)

            agin_r = agin_d[:].rearrange("(k p) f -> p k f", p=128)
            aggb_r = aggb_d[: c.TROW, :].rearrange("(k p) f -> p k f", p=128)
            aggv_r = aggb_d[c.TROW:, :].rearrange("(k p) f -> p k f", p=128)

            nreg = nc.gpsimd.alloc_register("nidx")
            _regval = [None]

            def set_nreg(v):
                if _regval[0] != v:
                    nc.gpsimd.reg_mov(nreg, v)
                    _regval[0] = v

            for l in range(3):
                K = c.IN if l == 0 else HID
                for k in range(c.NCHK):
                    if l == 0:
                        xt = work.tile([c.IN, 128], F32, tag="xt")
                        nc.sync.dma_start(xt[:], xT_d[:, k * 128:(k + 1) * 128])
                        lhsT = xt[:, :]
                    else:
                        lhsT = hT_sb[:K, k * 128:(k + 1) * 128]
                    ps = psA.tile([128, HID], F32, space="PSUM")
                    nc.tensor.matmul(ps[:], lhsT, W_sb[l][:K, :],
                                     start=True, stop=True)
                    nc.vector.tensor_scalar_mul(
                        stage_sb[:, k, :], ps[:], dinv_sb[:, k:k + 1])
                BK = 8
                for kk in range(cdiv(c.NCHK, BK)):
                    s = kk * BK
                    e = min(c.NCHK, s + BK)
                    nc.sync.dma_start(agin_r[:, s:e, :], stage_sb[:, s:e, :])
                    nc.sync.dma_start(aggb_r[:, s:e, :], stage_sb[:, s:e, :])
                # zero dump+virtual region
                nc.sync.dma_start(aggv_r[:], zero_sb[:])
                nc.gpsimd.collective_compute(
                    "AllGather", ALU.bypass,
                    replica_groups=rg, ins=[agin_d[:]], outs=[agout_d[:]])

                # software-pipelined: emit bin k+1's gathers before bin k's
                # scatter so the Pool engine never stalls on a gather DMA.
                pending = None  # (bin_sz, scol, msg)

                def flush_pending():
                    nonlocal pending
                    if pending is None:
                        return
                    p_sz, p_scol, p_msg = pending
                    si = work.tile([128, MAXMSG // 16], I16, tag="si",
                                   name=f"si_{l}_{p_scol}")
                    nc.sync.dma_start(
                        si[:, : p_sz // 16],
                        sidx_d[:, p_scol // 16:(p_scol + p_sz) // 16])
                    set_nreg(p_sz)
                    nc.gpsimd.dma_scatter_add(
                        aggb_d[:], p_msg[:, : p_sz // 128, :],
                        si[:, : p_sz // 16], p_sz, nreg, HID)
                    pending = None

                for bin_sz, scol, segs in CALLS:
                    msg = msgs.tile([128, MAXMSG // 128, HID], F32, tag="msg",
                                    name=f"msg_{l}_{scol}")
                    off = 0
                    for b, seg, col in segs:
                        gi = work.tile([128, MAXMSG // 16], I16, tag="gi",
                                       name=f"gi_{l}_{col}")
                        nc.sync.dma_start(
                            gi[:, : seg // 16],
                            gidx_d[:, col // 16:(col + seg) // 16])
                        set_nreg(seg)
                        nc.gpsimd.dma_gather(
                            msg[:, off // 128:(off + seg) // 128, :],
                            agout_d[b * c.SRCW:(b + 1) * c.SRCW, :],
                            gi[:, : seg // 16], seg, nreg, HID)
                        off += seg
                    flush_pending()
                    pending = (bin_sz, scol, msg)
                flush_pending()
                # merge virtual rows, deepest level first (reads aggb, so the
                # gather must follow all round scatters; keep these serial)
                col = TOT
                for vm in VM:
                    msg = msgs.tile([128, MAXMSG // 128, HID], F32, tag="msg",
                                    name=f"msgm_{l}_{col}")
                    gi = work.tile([128, MAXMSG // 16], I16, tag="gi",
                                   name=f"gim_{l}_{col}")
                    nc.sync.dma_start(
                        gi[:, : vm // 16], gidx_d[:, col // 16:(col + vm) // 16])
                    si = work.tile([128, MAXMSG // 16], I16, tag="si",
                                   name=f"sim_{l}_{col}")
                    nc.sync.dma_start(
                        si[:, : vm // 16], sidx_d[:, col // 16:(col + vm) // 16])
                    set_nreg(vm)
                    nc.gpsimd.dma_gather(
                        msg[:, : vm // 128, :], aggb_d[:],
                        gi[:, : vm // 16], vm, nreg, HID)
                    nc.gpsimd.dma_scatter_add(
                        aggb_d[:], msg[:, : vm // 128, :],
                        si[:, : vm // 16], vm, nreg, HID)
                    col += vm

                for kk in range(cdiv(c.NCHK, BK)):
                    s = kk * BK
                    e = min(c.NCHK, s + BK)
                    at = work.tile([128, BK, HID], F32, tag="at")
                    nc.sync.dma_start(at[:, : e - s, :], aggb_r[:, s:e, :])
                    for k in range(s, e):
                        v = work.tile([128, HID], F32, tag="v")
                        nc.vector.tensor_scalar_mul(
                            v[:], at[:, k - s, :], dinv_sb[:, k:k + 1])
                        if l < 2:
                            ps = psB.tile([HID, 128], F32, space="PSUM")
                            nc.tensor.transpose(ps[:], v[:], ident[:])
                            nc.scalar.activation(
                                hT_sb[:, k * 128:(k + 1) * 128], ps[:],
                                AF.Relu, bias=bcols_sb[:, l:l + 1])
                        else:
                            vb = work.tile([128, HID], F32, tag="vb")
                            nc.vector.tensor_add(vb[:], v[:], b3rep_sb[:])
                            nc.vector.tensor_relu(h3_sb[:, k, :], vb[:])

            # ---- mean pool ----
            pp = psA.tile([128, HID], F32, space="PSUM", tag="pool", bufs=1)
            for k in range(c.NCHK):
                oh = work.tile([128, 128], F32, tag="oh")
                nc.sync.dma_start(oh[:], oneh_d[k * 128:(k + 1) * 128, :])
                nc.tensor.matmul(
                    pp[:], oh[:], h3_sb[:, k, :],
                    start=(k == 0), stop=(k == c.NCHK - 1))
            pl = res.tile([128, HID], F32)
            nc.vector.tensor_copy(pl[:], pp[:])
            nc.sync.dma_start(plin_d[:], pl[:])
            nc.gpsimd.collective_compute(
                "AllReduce", ALU.add,
                replica_groups=rg, ins=[plin_d[:]], outs=[plout_d[:]])
            plr = res.tile([128, HID], F32)
            nc.sync.dma_start(plr[:], plout_d[:])
            plm = res.tile([128, HID], F32)
            nc.vector.tensor_scalar_mul(plm[:], plr[:], cinv_sb[:])
            psT = psB.tile([HID, 128], F32, space="PSUM", tag="pT", bufs=1)
            nc.tensor.transpose(psT[:], plm[:], ident[:])
            plT = res.tile([HID, 128], F32)
            nc.vector.tensor_copy(plT[:], psT[:])
            psC = psB.tile([G, OUT], F32, space="PSUM", tag="pC", bufs=1)
            nc.tensor.matmul(psC[:], plT[:, :G], Wc_sb[:], start=True, stop=True)
            lg = res.tile([G, OUT], F32)
            nc.vector.tensor_add(lg[:], psC[:, :], bcrep_sb[:G, :])
            mx = res.tile([G, 1], F32)
            nc.vector.tensor_reduce(mx[:], lg[:], mybir.AxisListType.X, ALU.max)
            lgs = res.tile([G, OUT], F32)
            nc.vector.tensor_scalar_sub(lgs[:], lg[:], mx[:])
            ex = res.tile([G, OUT], F32)
            nc.scalar.activation(ex[:], lgs[:], AF.Exp)
            sm = res.tile([G, 1], F32)
            nc.vector.tensor_reduce(sm[:], ex[:], mybir.AxisListType.X, ALU.add)
            ls = res.tile([G, 1], F32)
            nc.scalar.activation(ls[:], sm[:], AF.Ln)
            yt = res.tile([G, OUT], F32)
            nc.vector.tensor_scalar_sub(yt[:], lgs[:], ls[:])
            nc.sync.dma_start(y_d[:], yt[:])

    return nc


def _finalize(nc):
    nc.compile()
    fix_multiwait(nc)


def run(inputs, cfg, profile_dir=None):
    from concourse.bass_utils import run_bass_kernel_spmd

    in_maps, meta = prep(inputs, cfg)
    nc = build(cfg, meta)
    _finalize(nc)
    if profile_dir is not None:
        from trn_agent_boot.trn_boot import _ntff_profile_via_ctypes
        hook = _ntff_profile_via_ctypes("/opt/axon/libaxon_pjrt.so")
        with hook(profile_dir, [0]):
            res = run_bass_kernel_spmd(nc, in_maps, core_ids=list(range(cfg.C)))
    else:
        res = run_bass_kernel_spmd(nc, in_maps, core_ids=list(range(cfg.C)))
    return res.results[0]["y"]
# ---------------------------------------------------------------------------
N_NODES, N_EDGES, IN_DIM, HID_DIM, N_GRAPHS, OUT_DIM = 100_000, 1_600_000, 128, 64, 128, 3


def kernel(**inputs):
    import os
    cfg = Cfg(N_NODES, N_EDGES, IN_DIM, HID_DIM, N_GRAPHS, OUT_DIM, LV=20)
    out = run(inputs, cfg, profile_dir=os.environ.get("GNN_PROFILE_DIR"))
    return np.asarray(out, np.float32)



# revision 12
# speedup vs baseline: 1.5992x; 1.5992x over previous
"""3-layer GCN + mean-pool + classifier for Trainium2, SPMD on 8 NeuronCores.

Self-contained: kernel(**inputs) takes the full-size numpy inputs, does the
host-side graph partitioning, builds/compiles a Bass/Tile kernel, runs it on
cores 0-7 via run_bass_kernel_spmd, and returns the [128, 3] log-softmax
output.

Distribution: nodes are dst-sharded across the 8 cores. Per GCN layer each
core computes t' = dinv * (h @ W) for its shard (TensorE), casts to bf16 and
AllGathers the full node table, then per-edge messages are dma_gathered from
the HBM table (256 B rows). Aggregation runs entirely on TensorE: edges are
grouped into waves of 7 dst chunks (128 nodes each); per 128-edge message
tile the DVE builds a one-hot selection matrix S^T[e,d] = (dst_local[e]==d)
by iota-compare, and matmul(psum[cc], S^T, msgs) accumulates the scatter-add
in PSUM. No dma_scatter_add and no duplicate-dst rounds are needed (duplicate
dst within a tile just means two one-hot rows share a column). Gather calls
obey the HW limits (idx value < 8192 via 16 source buckets, <= 1024 idxs per
call); the per-(wave,bucket) call sizes are max-padded across cores so all 8
cores run an identical program (pads gather row 0 and select no column).
The GCN normalization deg^-1/2 (A+I) deg^-1/2 factorizes into a pre-scale of
t' and a post-scale of the aggregate; self-loops become an identity-matmul
seed of the PSUM accumulator. Mean-pooling is a one-hot matmul with an
AllReduce of per-core partials; the classifier + log_softmax run replicated.
"""
import sys

sys.path.insert(0, "/opt/trn_rl_repo")

import numpy as np
import ml_dtypes
import concourse.bacc as bacc
import concourse.mybir as mybir
import concourse.tile as tile

# ---------------------------------------------------------------------------
# Workarounds: this walrus build rejects >1 sync-wait per instruction.

import concourse.tile as _tile
import concourse.mybir as _mybir
from concourse.vector_clock import ScopedClock as _ScopedClock


def _split_waits_tail(nc, inst):
    si = inst.ins.sync_info
    if si is None or not si.on_wait or len(si.on_wait) <= 1:
        return
    waits = list(si.on_wait)
    inst.ins.sync_info = _mybir.SyncInfo(on_wait=[], on_update=list(si.on_update or []))
    for w in waits:
        nop = nc.sync.nop()
        nop.ins.sync_info = _mybir.SyncInfo(on_wait=[w], on_update=[])


def _drain_and_barrier(self, tick_clock, wait_clock):
    nc = self.nc
    probe = nc.sync.nop()
    wait_clock.add_sem_waits(probe.ins, _ScopedClock({None: tick_clock.global_clock}))
    _split_waits_tail(nc, probe)
    nc.sync.drain()
    nc.all_engine_barrier()
    assert self.sems is not None
    popped = nc._tile_sem_poison_stack.pop()
    assert popped is self._sem_poison
    nc.clear_and_free_semaphores(list(self.sems.allocated().values()))
    nc.all_engine_barrier()


_tile.TileContext._drain_and_barrier = _drain_and_barrier


def fix_multiwait(nc):
    """Rewrite every >1-wait instruction into wait-nops + 1-wait instruction."""
    for f in nc.m.functions:
        for blk in f.blocks:
            insts = blk.instructions            # live list (rust-backed)
            i = 0
            while i < len(insts):
                inst = insts[i]
                si = inst.sync_info
                if si is not None and si.on_wait and len(si.on_wait) > 1:
                    waits = list(si.on_wait)
                    eng = inst.engine
                    inst.sync_info = _mybir.SyncInfo(
                        on_wait=[waits[-1]], on_update=list(si.on_update or [])
                    )
                    for j, w in enumerate(waits[:-1]):
                        nop = nc.engines[eng].nop(hint="mwfix")
                        popped = False
                        for f2 in nc.m.functions:
                            for b2 in f2.blocks:
                                l2 = b2.instructions
                                if l2 and l2[-1].name == nop.ins.name:
                                    l2.pop()
                                    popped = True
                                    break
                            if popped:
                                break
                        assert popped, "could not relocate mwfix nop"
                        nop.ins.sync_info = _mybir.SyncInfo(on_wait=[w], on_update=[])
                        insts.insert(i + j, nop.ins)
                    i += len(waits) - 1
                i += 1


# ---------------------------------------------------------------------------

F32 = mybir.dt.float32
BF16 = mybir.dt.bfloat16
I16 = mybir.dt.int16
AF = mybir.ActivationFunctionType
ALU = mybir.AluOpType
NPBF = ml_dtypes.bfloat16


def cdiv(a, b):
    return (a + b - 1) // b


def rup(a, b):
    return cdiv(a, b) * b


class Cfg:
    def __init__(self, N, E, IN, HID, G, OUT, W=7):
        self.C = 8
        self.N, self.E, self.IN, self.HID, self.G, self.OUT = N, E, IN, HID, G, OUT
        assert N % self.C == 0
        self.NSH = N // self.C                  # 12500 nodes per core
        self.TROW = rup(self.NSH, 128)          # 12544
        self.NCHK = self.TROW // 128            # 98 dst chunks
        self.NBUK = 16
        assert (self.C * self.TROW) % self.NBUK == 0
        self.SRCW = self.C * self.TROW // self.NBUK   # 6272 rows per bucket
        assert self.SRCW <= 8191  # HW: gather idx value must fit 13 bits
        self.W = W                              # dst chunks per wave
        self.NW = cdiv(self.NCHK, W)
        self.MAXCALL = 1024                     # HW: gather <= 1024 idxs
        assert G <= 128


def _wrap_cols(a):
    """[n slots] (n % 16 == 0) -> wrapped [128, n // 16] int16."""
    w = a.reshape(-1, 16).T  # [16, n//16]
    return np.tile(w, (8, 1)).astype(np.int16)


def prep(inputs, cfg):
    c = cfg
    x = np.asarray(inputs["x"], np.float32)
    ei = np.asarray(inputs["edge_index"], np.int64)
    batch = np.asarray(inputs["batch"], np.int64)
    W1 = np.asarray(inputs["W1"], np.float32); b1 = np.asarray(inputs["b1"], np.float32)
    W2 = np.asarray(inputs["W2"], np.float32); b2 = np.asarray(inputs["b2"], np.float32)
    W3 = np.asarray(inputs["W3"], np.float32); b3 = np.asarray(inputs["b3"], np.float32)
    Wc = np.asarray(inputs["Wc"], np.float32); bc = np.asarray(inputs["bc"], np.float32)

    src = ei[0].astype(np.int64)
    dst = ei[1].astype(np.int64)
    deg = np.bincount(dst, minlength=c.N).astype(np.float32) + 1.0
    dinv = 1.0 / np.sqrt(deg)

    HID = c.HID
    W3p = np.zeros((HID, HID), np.float32); W3p[:, : W3.shape[1]] = W3
    b3p = np.zeros((HID,), np.float32); b3p[: b3.shape[0]] = b3
    Wcp = np.zeros((HID, c.OUT), np.float32); Wcp[: Wc.shape[0]] = Wc

    core_of = src // c.NSH
    trow_src = core_of * c.TROW + (src - core_of * c.NSH)
    buk = trow_src // c.SRCW
    gix = trow_src - buk * c.SRCW
    dcore = dst // c.NSH
    dloc = dst - dcore * c.NSH
    chk = dloc >> 7
    din = dloc & 127
    wav = chk // c.W

    NW, NB = c.NW, c.NBUK
    # --- SPMD-identical call structure: max segment size across cores
    SEG = np.zeros((NW, NB), np.int64)
    pc_data = []
    for ci in range(c.C):
        sel = np.flatnonzero(dcore == ci)
        o = sel[np.lexsort((din[sel], chk[sel], buk[sel], wav[sel]))]
        key = wav[o] * NB + buk[o]
        cnt = np.bincount(key, minlength=NW * NB)
        starts = np.concatenate([[0], np.cumsum(cnt)])
        pc_data.append((o, starts))
        SEG = np.maximum(SEG, cnt.reshape(NW, NB))
    SEGP = np.maximum(rup(SEG, 128), 128)
    assert SEGP.max() <= c.MAXCALL, f"segment overflow: {SEGP.max()}"

    calls = []      # dict(v, b, n, slot0)
    waves = []      # dict(chunks, calls, slot0, nslots, pair0, npairs)
    slot0 = 0
    for v in range(NW):
        wcalls = []
        ws0 = slot0
        for b in range(NB):
            n = int(SEGP[v, b])
            calls.append(dict(v=v, b=b, n=n, slot0=slot0))
            wcalls.append(len(calls) - 1)
            slot0 += n
        waves.append(dict(
            chunks=list(range(v * c.W, min((v + 1) * c.W, c.NCHK))),
            calls=wcalls, slot0=ws0, nslots=slot0 - ws0))
    TOT = slot0

    # --- per-core slot fill
    gs = np.zeros((c.C, TOT), np.int64)
    sch = np.full((c.C, TOT), -1, np.int64)
    sdi = np.zeros((c.C, TOT), np.int64)
    for ci in range(c.C):
        o, starts = pc_data[ci]
        for call in calls:
            k = call["v"] * NB + call["b"]
            e0, e1 = int(starts[k]), int(starts[k + 1])
            n = e1 - e0
            s0 = call["slot0"]
            idxs = o[e0:e1]
            gs[ci, s0:s0 + n] = gix[idxs]
            sch[ci, s0:s0 + n] = chk[idxs]
            sdi[ci, s0:s0 + n] = din[idxs]

    # --- pairs: per 128-slot tile, union of chunks present (across cores)
    pairs = []      # dict(call, t, cc, stop)
    for v, wm in enumerate(waves):
        p0 = len(pairs)
        for ic in wm["calls"]:
            call = calls[ic]
            for t in range(call["n"] // 128):
                sl = slice(call["slot0"] + t * 128, call["slot0"] + (t + 1) * 128)
                u = np.unique(sch[:, sl])
                u = u[u >= 0]
                for cc in u:
                    pairs.append(dict(call=ic, t=t, cc=int(cc), stop=False))
        last = {}
        for j in range(p0, len(pairs)):
            last[pairs[j]["cc"]] = j
        for j in range(p0, len(pairs)):
            pairs[j]["stop"] = last[pairs[j]["cc"]] == j
        for cc in wm["chunks"]:
            assert cc in last, f"chunk {cc} has no edges in wave {v}"
        wm["pair0"] = p0
        wm["npairs"] = len(pairs) - p0
    NPAIRS = len(pairs)

    # --- per-core one-hot select columns
    dsel = np.full((c.C, 128, NPAIRS), 200.0, np.float32)
    for j, pr in enumerate(pairs):
        call = calls[pr["call"]]
        sl = slice(call["slot0"] + pr["t"] * 128, call["slot0"] + (pr["t"] + 1) * 128)
        m = sch[:, sl] == pr["cc"]
        dsel[:, :, j] = np.where(m, sdi[:, sl], 200).astype(np.float32)

    cnt = np.bincount(batch, minlength=c.G).astype(np.float32)
    cntinv = (1.0 / np.maximum(cnt, 1.0)).astype(np.float32)

    iota_row = np.tile(np.arange(128, dtype=np.float32), (128, 1)).astype(NPBF)
    identb = np.eye(128, dtype=np.float32).astype(NPBF)

    in_maps = []
    for ci in range(c.C):
        lo, hi = ci * c.NSH, (ci + 1) * c.NSH
        xT = np.zeros((c.IN, c.TROW), np.float32)
        xT[:, : c.NSH] = x[lo:hi].T
        dv = np.zeros((c.TROW,), np.float32)
        dv[: c.NSH] = dinv[lo:hi]
        dinv2d = dv.reshape(c.NCHK, 128).T.copy()

        oneh = np.zeros((c.TROW, 128), np.float32)
        oneh[np.arange(c.NSH), batch[lo:hi]] = 1.0

        bcols = np.stack([b1, b2, b3p], axis=1)
        b3rep = np.tile(b3p[None, :], (128, 1))
        bcrep = np.tile(bc[None, :], (128, 1))
        cinv = np.zeros((128, 1), np.float32)
        cinv[: c.G, 0] = cntinv

        in_maps.append(dict(
            xT=xT, dinv2d=dinv2d,
            gidx=_wrap_cols(gs[ci]),
            dsel=dsel[ci],
            oneh=oneh.astype(NPBF),
            W1d=W1, W2d=W2.astype(NPBF), W3d=W3p.astype(NPBF),
            bcols=bcols, b3rep=b3rep,
            Wcp=Wcp, bcrep=bcrep, cinv=cinv,
            iota=iota_row, identb=identb,
        ))

    meta = dict(calls=calls, waves=waves, pairs=pairs, TOT=TOT, NPAIRS=NPAIRS)
    return in_maps, meta


def build(cfg, meta):
    c = cfg
    HID, G, OUT = c.HID, c.G, c.OUT
    calls, waves, pairs = meta["calls"], meta["waves"], meta["pairs"]
    TOT, NPAIRS = meta["TOT"], meta["NPAIRS"]
    IDXC = max(wm["nslots"] // 16 for wm in waves)
    DSELC = max(wm["npairs"] for wm in waves)
    MAXT = c.MAXCALL // 128     # 8 tiles per call

    nc = bacc.Bacc("TRN2", num_devices=c.C, dynamic_dma_scratch_size=65536)

    def ein(name, shape, dt=F32):
        return nc.dram_tensor(name, shape, dt, kind="ExternalInput")

    xT_d = ein("xT", [c.IN, c.TROW])
    dinv_d = ein("dinv2d", [128, c.NCHK])
    gidx_d = ein("gidx", [128, TOT // 16], I16)
    dsel_d = ein("dsel", [128, NPAIRS])
    oneh_d = ein("oneh", [c.TROW, 128], BF16)
    W1_d = ein("W1d", [c.IN, HID])
    W2_d = ein("W2d", [HID, HID], BF16)
    W3_d = ein("W3d", [HID, HID], BF16)
    bcols_d = ein("bcols", [HID, 3])
    b3rep_d = ein("b3rep", [128, HID])
    Wc_d = ein("Wcp", [HID, OUT])
    bcrep_d = ein("bcrep", [128, OUT])
    cinv_d = ein("cinv", [128, 1])
    iota_d = ein("iota", [128, 128], BF16)
    identb_d = ein("identb", [128, 128], BF16)

    agin_d = nc.dram_tensor("agin", [c.TROW, 128], BF16, kind="Internal")
    agout_d = nc.dram_tensor(
        "agout", [c.C * c.TROW, 128], BF16, kind="Internal", addr_space="Shared")
    plin_d = nc.dram_tensor("plin", [128, HID], F32, kind="Internal")
    plout_d = nc.dram_tensor(
        "plout", [128, HID], F32, kind="Internal", addr_space="Shared")
    y_d = nc.dram_tensor("y", [G, OUT], F32, kind="ExternalOutput")

    rg = [list(range(c.C))]

    with tile.TileContext(nc) as tc:
        with (
            tc.tile_pool(name="res", bufs=1) as res,
            tc.tile_pool(name="stage", bufs=1) as stpool,
            tc.tile_pool(name="xt", bufs=3) as xtpool,
            tc.tile_pool(name="msg", bufs=28) as msgpool,
            tc.tile_pool(name="widx", bufs=2) as widxpool,
            tc.tile_pool(name="wdsel", bufs=2) as dselpool,
            tc.tile_pool(name="sel", bufs=6) as selpool,
            tc.tile_pool(name="fin", bufs=4) as finpool,
            tc.tile_pool(name="oh", bufs=2) as ohpool,
            tc.tile_pool(name="psA", bufs=2, space="PSUM") as psA,
            tc.tile_pool(name="psT", bufs=2, space="PSUM") as psT,
            tc.tile_pool(name="psW", bufs=2, space="PSUM") as psW,
        ):
            from concourse.masks import make_identity
            ident = res.tile([128, 128], F32)
            make_identity(nc, ident[:])
            identb_sb = res.tile([128, 128], BF16)
            nc.sync.dma_start(identb_sb[:], identb_d[:])
            iota_sb = res.tile([128, 128], BF16)
            nc.sync.dma_start(iota_sb[:], iota_d[:])
            dinv_sb = res.tile([128, c.NCHK], F32)
            nc.sync.dma_start(dinv_sb[:], dinv_d[:])
            W1_sb = res.tile([c.IN, HID], F32, name="w1")
            nc.sync.dma_start(W1_sb[:], W1_d[:])
            W2_sb = res.tile([HID, HID], BF16, name="w2")
            nc.sync.dma_start(W2_sb[:], W2_d[:])
            W3_sb = res.tile([HID, HID], BF16, name="w3")
            nc.sync.dma_start(W3_sb[:], W3_d[:])
            bcols_sb = res.tile([HID, 3], F32)
            nc.sync.dma_start(bcols_sb[:], bcols_d[:])
            b3rep_sb = res.tile([128, HID], F32)
            nc.sync.dma_start(b3rep_sb[:], b3rep_d[:])
            Wc_sb = res.tile([HID, OUT], F32)
            nc.sync.dma_start(Wc_sb[:], Wc_d[:])
            bcrep_sb = res.tile([128, OUT], F32)
            nc.sync.dma_start(bcrep_sb[:], bcrep_d[:])
            cinv_sb = res.tile([128, 1], F32)
            nc.sync.dma_start(cinv_sb[:], cinv_d[:])

            stage_sb = stpool.tile([128, c.NCHK, 128], BF16)
            hT_sb = stpool.tile([HID, c.TROW], BF16)
            h3_sb = stpool.tile([128, c.NCHK, HID], BF16)
            # message cols 64:128 of the table are never matmul'd but are
            # gathered; zero once so they are defined.
            nc.vector.memset(stage_sb[:, :, HID:], 0.0)

            agin_r = agin_d[:].rearrange("(k p) f -> p k f", p=128)
            oneh_r = oneh_d[:].rearrange("(k p) f -> p k f", p=128)

            nreg = nc.gpsimd.alloc_register("nidx")
            _regval = [None]

            def set_nreg(v):
                if _regval[0] != v:
                    nc.gpsimd.reg_mov(nreg, v)
                    _regval[0] = v

            BK = 8
            for l in range(3):
                # ---- transform: stage = dinv * (h @ W) ----
                for kk in range(cdiv(c.NCHK, BK)):
                    s = kk * BK
                    e = min(c.NCHK, s + BK)
                    if l == 0:
                        xt = xtpool.tile([c.IN, BK * 128], F32, tag="xt")
                        nc.sync.dma_start(
                            xt[:, : (e - s) * 128],
                            xT_d[:, s * 128:e * 128])
                    for k in range(s, e):
                        if l == 0:
                            lhsT = xt[:, (k - s) * 128:(k - s + 1) * 128]
                            Wl = W1_sb
                        else:
                            lhsT = hT_sb[:, k * 128:(k + 1) * 128]
                            Wl = W2_sb if l == 1 else W3_sb
                        ps = psA.tile([128, HID], F32, space="PSUM")
                        nc.tensor.matmul(ps[:], lhsT, Wl[:], start=True, stop=True)
                        nc.vector.tensor_scalar_mul(
                            stage_sb[:, k, :HID], ps[:], dinv_sb[:, k:k + 1])
                for kk in range(cdiv(c.NCHK, BK)):
                    s = kk * BK
                    e = min(c.NCHK, s + BK)
                    nc.sync.dma_start(agin_r[:, s:e, :], stage_sb[:, s:e, :])
                nc.gpsimd.collective_compute(
                    "AllGather", ALU.bypass,
                    replica_groups=rg, ins=[agin_d[:]], outs=[agout_d[:]])

                # ---- waves: gather + one-hot matmul aggregation ----
                for v, wm in enumerate(waves):
                    widx = widxpool.tile([128, IDXC], I16, tag="widx",
                                         name=f"widx_{l}_{v}")
                    col0 = wm["slot0"] // 16
                    ncol = wm["nslots"] // 16
                    nc.sync.dma_start(widx[:, :ncol], gidx_d[:, col0:col0 + ncol])
                    wdsel = dselpool.tile([128, DSELC], F32, tag="wdsel",
                                          name=f"wdsel_{l}_{v}")
                    nc.sync.dma_start(
                        wdsel[:, : wm["npairs"]],
                        dsel_d[:, wm["pair0"]:wm["pair0"] + wm["npairs"]])

                    # seed accumulators with self-loop term t' (all chunk
                    # accumulators of a wave share one PSUM bank tile)
                    ppw = psW.tile([128, c.W * HID], F32, space="PSUM",
                                   tag="pp", name=f"pp_{l}_{v}")
                    pp = {}
                    for i, cc in enumerate(wm["chunks"]):
                        pp[cc] = ppw[:, i * HID:(i + 1) * HID]
                        nc.tensor.matmul(
                            pp[cc], identb_sb[:], stage_sb[:, cc, :HID],
                            start=True, stop=False)

                    # gathers (one per bucket)
                    mtiles = {}
                    for ic in wm["calls"]:
                        call = calls[ic]
                        n = call["n"]
                        b = call["b"]
                        mt = msgpool.tile([128, MAXT, 128], BF16, tag="msg",
                                          name=f"msg_{l}_{ic}")
                        lo = (call["slot0"] - wm["slot0"]) // 16
                        set_nreg(n)
                        nc.gpsimd.dma_gather(
                            mt[:, : n // 128, :],
                            agout_d[b * c.SRCW:(b + 1) * c.SRCW, :],
                            widx[:, lo:lo + n // 16], n, nreg, 128)
                        mtiles[ic] = mt

                    # one-hot matmul accumulation
                    for j in range(wm["pair0"], wm["pair0"] + wm["npairs"]):
                        pr = pairs[j]
                        st = selpool.tile([128, 128], BF16, tag="sel",
                                          name=f"sel_{l}_{j}")
                        nc.vector.tensor_scalar(
                            st[:], iota_sb[:],
                            wdsel[:, j - wm["pair0"]:j - wm["pair0"] + 1],
                            None, ALU.is_equal)
                        nc.tensor.matmul(
                            pp[pr["cc"]], st[:],
                            mtiles[pr["call"]][:, pr["t"], :HID],
                            start=False, stop=pr["stop"])

                    # finalize chunks of this wave
                    for cc in wm["chunks"]:
                        vv = finpool.tile([128, HID], F32, tag="v")
                        nc.vector.tensor_scalar_mul(
                            vv[:], pp[cc], dinv_sb[:, cc:cc + 1])
                        if l < 2:
                            pt = psT.tile([HID, 128], F32, space="PSUM", tag="tp")
                            nc.tensor.transpose(pt[:], vv[:], ident[:])
                            nc.scalar.activation(
                                hT_sb[:, cc * 128:(cc + 1) * 128], pt[:],
                                AF.Relu, bias=bcols_sb[:, l:l + 1])
                        else:
                            vb = finpool.tile([128, HID], F32, tag="vb")
                            nc.vector.tensor_add(vb[:], vv[:], b3rep_sb[:])
                            nc.vector.tensor_relu(h3_sb[:, cc, :], vb[:])

            # ---- mean pool ----
            pq = psA.tile([128, HID], F32, space="PSUM", tag="pool", bufs=1)
            for kk in range(cdiv(c.NCHK, BK)):
                s = kk * BK
                e = min(c.NCHK, s + BK)
                oh = ohpool.tile([128, BK, 128], BF16, tag="oh")
                nc.sync.dma_start(oh[:, : e - s, :], oneh_r[:, s:e, :])
                for k in range(s, e):
                    nc.tensor.matmul(
                        pq[:], oh[:, k - s, :], h3_sb[:, k, :],
                        start=(k == 0), stop=(k == c.NCHK - 1))
            pl = res.tile([128, HID], F32)
            nc.vector.tensor_copy(pl[:], pq[:])
            nc.sync.dma_start(plin_d[:], pl[:])
            nc.gpsimd.collective_compute(
                "AllReduce", ALU.add,
                replica_groups=rg, ins=[plin_d[:]], outs=[plout_d[:]])
            plr = res.tile([128, HID], F32)
            nc.sync.dma_start(plr[:], plout_d[:])
            plm = res.tile([128, HID], F32)
            nc.vector.tensor_scalar_mul(plm[:], plr[:], cinv_sb[:])
            pst = psT.tile([HID, 128], F32, space="PSUM", tag="tp")
            nc.tensor.transpose(pst[:], plm[:], ident[:])
            plT = res.tile([HID, 128], F32)
            nc.vector.tensor_copy(plT[:], pst[:])
            psC = psT.tile([G, OUT], F32, space="PSUM", tag="pC", bufs=1)
            nc.tensor.matmul(psC[:], plT[:, :G], Wc_sb[:], start=True, stop=True)
            lg = res.tile([G, OUT], F32)
            nc.vector.tensor_add(lg[:], psC[:, :], bcrep_sb[:G, :])
            mx = res.tile([G, 1], F32)
            nc.vector.tensor_reduce(mx[:], lg[:], mybir.AxisListType.X, ALU.max)
            lgs = res.tile([G, OUT], F32)
            nc.vector.tensor_scalar_sub(lgs[:], lg[:], mx[:])
            ex = res.tile([G, OUT], F32)
            nc.scalar.activation(ex[:], lgs[:], AF.Exp)
            sm = res.tile([G, 1], F32)
            nc.vector.tensor_reduce(sm[:], ex[:], mybir.AxisListType.X, ALU.add)
            ls = res.tile([G, 1], F32)
            nc.scalar.activation(ls[:], sm[:], AF.Ln)
            yt = res.tile([G, OUT], F32)
            nc.vector.tensor_scalar_sub(yt[:], lgs[:], ls[:])
            nc.sync.dma_start(y_d[:], yt[:])

    return nc


def _finalize(nc):
    nc.compile()
    fix_multiwait(nc)


def run(inputs, cfg, profile_dir=None):
    from concourse.bass_utils import run_bass_kernel_spmd

    in_maps, meta = prep(inputs, cfg)
    nc = build(cfg, meta)
    _finalize(nc)
    if profile_dir is not None:
        from trn_agent_boot.trn_boot import _ntff_profile_via_ctypes
        hook = _ntff_profile_via_ctypes("/opt/axon/libaxon_pjrt.so")
        with hook(profile_dir, [0]):
            res = run_bass_kernel_spmd(nc, in_maps, core_ids=list(range(cfg.C)))
    else:
        res = run_bass_kernel_spmd(nc, in_maps, core_ids=list(range(cfg.C)))
    return res.results[0]["y"]


# ---------------------------------------------------------------------------
N_NODES, N_EDGES, IN_DIM, HID_DIM, N_GRAPHS, OUT_DIM = 100_000, 1_600_000, 128, 64, 128, 3


def kernel(**inputs):
    import os
    cfg = Cfg(N_NODES, N_EDGES, IN_DIM, HID_DIM, N_GRAPHS, OUT_DIM, W=7)
    out = run(inputs, cfg, profile_dir=os.environ.get("GNN_PROFILE_DIR"))
    return np.asarray(out, np.float32)


# revision 14
# speedup vs baseline: 1.7246x; 1.0784x over previous
"""3-layer GCN + mean-pool + classifier for Trainium2, SPMD on 8 NeuronCores.

Self-contained: kernel(**inputs) takes the full-size numpy inputs, does the
host-side graph partitioning, builds/compiles a Bass/Tile kernel, runs it on
cores 0-7 via run_bass_kernel_spmd, and returns the [128, 3] log-softmax
output.

Distribution: nodes are dst-sharded across the 8 cores. Per GCN layer each
core computes t' = dinv * (h @ W) for its shard (TensorE), casts to bf16 and
AllGathers the full node table, then per-edge messages are dma_gathered from
the HBM table (256 B rows). Aggregation runs entirely on TensorE: edges are
grouped into waves of 7 dst chunks (128 nodes each); per 128-edge message
tile the DVE builds a one-hot selection matrix S^T[e,d] = (dst_local[e]==d)
by iota-compare, and matmul(psum[cc], S^T, msgs) accumulates the scatter-add
in PSUM. No dma_scatter_add and no duplicate-dst rounds are needed (duplicate
dst within a tile just means two one-hot rows share a column). Gather calls
obey the HW limits (idx value < 8192 via 16 source buckets, <= 1024 idxs per
call); the per-(wave,bucket) call sizes are max-padded across cores so all 8
cores run an identical program (pads gather row 0 and select no column).
The GCN normalization deg^-1/2 (A+I) deg^-1/2 factorizes into a pre-scale of
t' and a post-scale of the aggregate; self-loops become an identity-matmul
seed of the PSUM accumulator. Mean-pooling is a one-hot matmul with an
AllReduce of per-core partials; the classifier + log_softmax run replicated.
"""
import sys

sys.path.insert(0, "/opt/trn_rl_repo")

import numpy as np
import ml_dtypes
import concourse.bacc as bacc
import concourse.mybir as mybir
import concourse.tile as tile

# ---------------------------------------------------------------------------
# Workarounds: this walrus build rejects >1 sync-wait per instruction.

import concourse.tile as _tile
import concourse.mybir as _mybir
from concourse.vector_clock import ScopedClock as _ScopedClock


def _split_waits_tail(nc, inst):
    si = inst.ins.sync_info
    if si is None or not si.on_wait or len(si.on_wait) <= 1:
        return
    waits = list(si.on_wait)
    inst.ins.sync_info = _mybir.SyncInfo(on_wait=[], on_update=list(si.on_update or []))
    for w in waits:
        nop = nc.sync.nop()
        nop.ins.sync_info = _mybir.SyncInfo(on_wait=[w], on_update=[])


def _drain_and_barrier(self, tick_clock, wait_clock):
    nc = self.nc
    probe = nc.sync.nop()
    wait_clock.add_sem_waits(probe.ins, _ScopedClock({None: tick_clock.global_clock}))
    _split_waits_tail(nc, probe)
    nc.sync.drain()
    nc.all_engine_barrier()
    assert self.sems is not None
    popped = nc._tile_sem_poison_stack.pop()
    assert popped is self._sem_poison
    nc.clear_and_free_semaphores(list(self.sems.allocated().values()))
    nc.all_engine_barrier()


_tile.TileContext._drain_and_barrier = _drain_and_barrier


def fix_multiwait(nc):
    """Rewrite every >1-wait instruction into wait-nops + 1-wait instruction."""
    for f in nc.m.functions:
        for blk in f.blocks:
            insts = blk.instructions            # live list (rust-backed)
            i = 0
            while i < len(insts):
                inst = insts[i]
                si = inst.sync_info
                if si is not None and si.on_wait and len(si.on_wait) > 1:
                    waits = list(si.on_wait)
                    eng = inst.engine
                    inst.sync_info = _mybir.SyncInfo(
                        on_wait=[waits[-1]], on_update=list(si.on_update or [])
                    )
                    for j, w in enumerate(waits[:-1]):
                        nop = nc.engines[eng].nop(hint="mwfix")
                        popped = False
                        for f2 in nc.m.functions:
                            for b2 in f2.blocks:
                                l2 = b2.instructions
                                if l2 and l2[-1].name == nop.ins.name:
                                    l2.pop()
                                    popped = True
                                    break
                            if popped:
                                break
                        assert popped, "could not relocate mwfix nop"
                        nop.ins.sync_info = _mybir.SyncInfo(on_wait=[w], on_update=[])
                        insts.insert(i + j, nop.ins)
                    i += len(waits) - 1
                i += 1


# ---------------------------------------------------------------------------

F32 = mybir.dt.float32
BF16 = mybir.dt.bfloat16
I16 = mybir.dt.int16
AF = mybir.ActivationFunctionType
ALU = mybir.AluOpType
NPBF = ml_dtypes.bfloat16


def cdiv(a, b):
    return (a + b - 1) // b


def rup(a, b):
    return cdiv(a, b) * b


class Cfg:
    def __init__(self, N, E, IN, HID, G, OUT, W=7):
        self.C = 8
        self.N, self.E, self.IN, self.HID, self.G, self.OUT = N, E, IN, HID, G, OUT
        assert N % self.C == 0
        self.NSH = N // self.C                  # 12500 nodes per core
        self.TROW = rup(self.NSH, 128)          # 12544
        self.NCHK = self.TROW // 128            # 98 dst chunks
        self.NBUK = 16
        assert (self.C * self.TROW) % self.NBUK == 0
        self.SRCW = self.C * self.TROW // self.NBUK   # 6272 rows per bucket
        assert self.SRCW <= 8191  # HW: gather idx value must fit 13 bits
        self.W = W                              # dst chunks per wave
        self.NW = cdiv(self.NCHK, W)
        self.MAXCALL = 1024                     # HW: gather <= 1024 idxs
        assert G <= 128


def _wrap_cols(a):
    """[n slots] (n % 16 == 0) -> wrapped [128, n // 16] int16."""
    w = a.reshape(-1, 16).T  # [16, n//16]
    return np.tile(w, (8, 1)).astype(np.int16)


def prep(inputs, cfg):
    c = cfg
    x = np.asarray(inputs["x"], np.float32)
    ei = np.asarray(inputs["edge_index"], np.int64)
    batch = np.asarray(inputs["batch"], np.int64)
    W1 = np.asarray(inputs["W1"], np.float32); b1 = np.asarray(inputs["b1"], np.float32)
    W2 = np.asarray(inputs["W2"], np.float32); b2 = np.asarray(inputs["b2"], np.float32)
    W3 = np.asarray(inputs["W3"], np.float32); b3 = np.asarray(inputs["b3"], np.float32)
    Wc = np.asarray(inputs["Wc"], np.float32); bc = np.asarray(inputs["bc"], np.float32)

    src = ei[0].astype(np.int64)
    dst = ei[1].astype(np.int64)
    deg = np.bincount(dst, minlength=c.N).astype(np.float32) + 1.0
    dinv = 1.0 / np.sqrt(deg)

    HID = c.HID
    W3p = np.zeros((HID, HID), np.float32); W3p[:, : W3.shape[1]] = W3
    b3p = np.zeros((HID,), np.float32); b3p[: b3.shape[0]] = b3
    Wcp = np.zeros((HID, c.OUT), np.float32); Wcp[: Wc.shape[0]] = Wc

    core_of = src // c.NSH
    trow_src = core_of * c.TROW + (src - core_of * c.NSH)
    buk = trow_src // c.SRCW
    gix = trow_src - buk * c.SRCW
    dcore = dst // c.NSH
    dloc = dst - dcore * c.NSH
    chk = dloc >> 7
    din = dloc & 127
    wav = chk // c.W

    NW, NB = c.NW, c.NBUK
    # --- SPMD-identical call structure: max segment size across cores
    SEG = np.zeros((NW, NB), np.int64)
    pc_data = []
    for ci in range(c.C):
        sel = np.flatnonzero(dcore == ci)
        o = sel[np.lexsort((din[sel], chk[sel], buk[sel], wav[sel]))]
        key = wav[o] * NB + buk[o]
        cnt = np.bincount(key, minlength=NW * NB)
        starts = np.concatenate([[0], np.cumsum(cnt)])
        pc_data.append((o, starts))
        SEG = np.maximum(SEG, cnt.reshape(NW, NB))
    SEGP = np.maximum(rup(SEG, 128), 128)
    assert SEGP.max() <= c.MAXCALL, f"segment overflow: {SEGP.max()}"

    calls = []      # dict(v, b, n, slot0)
    waves = []      # dict(chunks, calls, slot0, nslots, pair0, npairs)
    slot0 = 0
    for v in range(NW):
        wcalls = []
        ws0 = slot0
        for b in range(NB):
            n = int(SEGP[v, b])
            calls.append(dict(v=v, b=b, n=n, slot0=slot0))
            wcalls.append(len(calls) - 1)
            slot0 += n
        waves.append(dict(
            chunks=list(range(v * c.W, min((v + 1) * c.W, c.NCHK))),
            calls=wcalls, slot0=ws0, nslots=slot0 - ws0))
    TOT = slot0

    # --- per-core slot fill
    gs = np.zeros((c.C, TOT), np.int64)
    sch = np.full((c.C, TOT), -1, np.int64)
    sdi = np.zeros((c.C, TOT), np.int64)
    for ci in range(c.C):
        o, starts = pc_data[ci]
        for call in calls:
            k = call["v"] * NB + call["b"]
            e0, e1 = int(starts[k]), int(starts[k + 1])
            n = e1 - e0
            s0 = call["slot0"]
            idxs = o[e0:e1]
            gs[ci, s0:s0 + n] = gix[idxs]
            sch[ci, s0:s0 + n] = chk[idxs]
            sdi[ci, s0:s0 + n] = din[idxs]

    # --- pairs: per 128-slot tile, union of chunks present (across cores)
    pairs = []      # dict(call, t, cc, stop)
    for v, wm in enumerate(waves):
        p0 = len(pairs)
        for ic in wm["calls"]:
            call = calls[ic]
            for t in range(call["n"] // 128):
                sl = slice(call["slot0"] + t * 128, call["slot0"] + (t + 1) * 128)
                u = np.unique(sch[:, sl])
                u = u[u >= 0]
                for cc in u:
                    pairs.append(dict(call=ic, t=t, cc=int(cc), stop=False))
        last = {}
        for j in range(p0, len(pairs)):
            last[pairs[j]["cc"]] = j
        for j in range(p0, len(pairs)):
            pairs[j]["stop"] = last[pairs[j]["cc"]] == j
        for cc in wm["chunks"]:
            assert cc in last, f"chunk {cc} has no edges in wave {v}"
        wm["pair0"] = p0
        wm["npairs"] = len(pairs) - p0
    NPAIRS = len(pairs)

    # --- per-core one-hot select columns
    dsel = np.full((c.C, 128, NPAIRS), 200.0, np.float32)
    for j, pr in enumerate(pairs):
        call = calls[pr["call"]]
        sl = slice(call["slot0"] + pr["t"] * 128, call["slot0"] + (pr["t"] + 1) * 128)
        m = sch[:, sl] == pr["cc"]
        dsel[:, :, j] = np.where(m, sdi[:, sl], 200).astype(np.float32)

    cnt = np.bincount(batch, minlength=c.G).astype(np.float32)
    cntinv = (1.0 / np.maximum(cnt, 1.0)).astype(np.float32)

    iota_row = np.tile(np.arange(128, dtype=np.float32), (128, 1)).astype(NPBF)
    identb = np.eye(128, dtype=np.float32).astype(NPBF)

    in_maps = []
    for ci in range(c.C):
        lo, hi = ci * c.NSH, (ci + 1) * c.NSH
        xT = np.zeros((c.IN, c.TROW), np.float32)
        xT[:, : c.NSH] = x[lo:hi].T
        dv = np.zeros((c.TROW,), np.float32)
        dv[: c.NSH] = dinv[lo:hi]
        dinv2d = dv.reshape(c.NCHK, 128).T.copy()

        oneh = np.zeros((c.TROW, 128), np.float32)
        oneh[np.arange(c.NSH), batch[lo:hi]] = 1.0

        bcols = np.stack([b1, b2, b3p], axis=1)
        b3rep = np.tile(b3p[None, :], (128, 1))
        bcrep = np.tile(bc[None, :], (128, 1))
        cinv = np.zeros((128, 1), np.float32)
        cinv[: c.G, 0] = cntinv

        in_maps.append(dict(
            xT=xT, dinv2d=dinv2d,
            gidx=_wrap_cols(gs[ci]),
            dsel=dsel[ci],
            oneh=oneh.astype(NPBF),
            W1d=W1, W2d=W2.astype(NPBF), W3d=W3p.astype(NPBF),
            bcols=bcols, b3rep=b3rep,
            Wcp=Wcp, bcrep=bcrep, cinv=cinv,
            iota=iota_row, identb=identb,
        ))

    meta = dict(calls=calls, waves=waves, pairs=pairs, TOT=TOT, NPAIRS=NPAIRS)
    return in_maps, meta


def build(cfg, meta):
    c = cfg
    HID, G, OUT = c.HID, c.G, c.OUT
    calls, waves, pairs = meta["calls"], meta["waves"], meta["pairs"]
    TOT, NPAIRS = meta["TOT"], meta["NPAIRS"]
    IDXC = max(wm["nslots"] // 16 for wm in waves)
    DSELC = max(wm["npairs"] for wm in waves)
    MAXT = c.MAXCALL // 128     # 8 tiles per call

    nc = bacc.Bacc("TRN2", num_devices=c.C, dynamic_dma_scratch_size=65536,
                   num_swdge_queues=4)

    def ein(name, shape, dt=F32):
        return nc.dram_tensor(name, shape, dt, kind="ExternalInput")

    xT_d = ein("xT", [c.IN, c.TROW])
    dinv_d = ein("dinv2d", [128, c.NCHK])
    gidx_d = ein("gidx", [128, TOT // 16], I16)
    dsel_d = ein("dsel", [128, NPAIRS])
    oneh_d = ein("oneh", [c.TROW, 128], BF16)
    W1_d = ein("W1d", [c.IN, HID])
    W2_d = ein("W2d", [HID, HID], BF16)
    W3_d = ein("W3d", [HID, HID], BF16)
    bcols_d = ein("bcols", [HID, 3])
    b3rep_d = ein("b3rep", [128, HID])
    Wc_d = ein("Wcp", [HID, OUT])
    bcrep_d = ein("bcrep", [128, OUT])
    cinv_d = ein("cinv", [128, 1])
    iota_d = ein("iota", [128, 128], BF16)
    identb_d = ein("identb", [128, 128], BF16)

    agin_d = nc.dram_tensor("agin", [c.TROW, 128], BF16, kind="Internal")
    agout_d = nc.dram_tensor(
        "agout", [c.C * c.TROW, 128], BF16, kind="Internal", addr_space="Shared")
    plin_d = nc.dram_tensor("plin", [128, HID], F32, kind="Internal")
    plout_d = nc.dram_tensor(
        "plout", [128, HID], F32, kind="Internal", addr_space="Shared")
    y_d = nc.dram_tensor("y", [G, OUT], F32, kind="ExternalOutput")

    rg = [list(range(c.C))]

    with tile.TileContext(nc) as tc:
        with (
            tc.tile_pool(name="res", bufs=1) as res,
            tc.tile_pool(name="stage", bufs=1) as stpool,
            tc.tile_pool(name="xt", bufs=3) as xtpool,
            tc.tile_pool(name="msg", bufs=28) as msgpool,
            tc.tile_pool(name="widx", bufs=2) as widxpool,
            tc.tile_pool(name="wdsel", bufs=2) as dselpool,
            tc.tile_pool(name="sel", bufs=6) as selpool,
            tc.tile_pool(name="fin", bufs=4) as finpool,
            tc.tile_pool(name="oh", bufs=2) as ohpool,
            tc.tile_pool(name="psA", bufs=2, space="PSUM") as psA,
            tc.tile_pool(name="psT", bufs=2, space="PSUM") as psT,
            tc.tile_pool(name="psW", bufs=2, space="PSUM") as psW,
        ):
            from concourse.masks import make_identity
            ident = res.tile([128, 128], F32)
            make_identity(nc, ident[:])
            identb_sb = res.tile([128, 128], BF16)
            nc.sync.dma_start(identb_sb[:], identb_d[:])
            iota_sb = res.tile([128, 128], BF16)
            nc.sync.dma_start(iota_sb[:], iota_d[:])
            dinv_sb = res.tile([128, c.NCHK], F32)
            nc.sync.dma_start(dinv_sb[:], dinv_d[:])
            W1_sb = res.tile([c.IN, HID], F32, name="w1")
            nc.sync.dma_start(W1_sb[:], W1_d[:])
            W2_sb = res.tile([HID, HID], BF16, name="w2")
            nc.sync.dma_start(W2_sb[:], W2_d[:])
            W3_sb = res.tile([HID, HID], BF16, name="w3")
            nc.sync.dma_start(W3_sb[:], W3_d[:])
            bcols_sb = res.tile([HID, 3], F32)
            nc.sync.dma_start(bcols_sb[:], bcols_d[:])
            b3rep_sb = res.tile([128, HID], F32)
            nc.sync.dma_start(b3rep_sb[:], b3rep_d[:])
            Wc_sb = res.tile([HID, OUT], F32)
            nc.sync.dma_start(Wc_sb[:], Wc_d[:])
            bcrep_sb = res.tile([128, OUT], F32)
            nc.sync.dma_start(bcrep_sb[:], bcrep_d[:])
            cinv_sb = res.tile([128, 1], F32)
            nc.sync.dma_start(cinv_sb[:], cinv_d[:])

            stage_sb = stpool.tile([128, c.NCHK, 128], BF16)
            hT_sb = stpool.tile([HID, c.TROW], BF16)
            h3_sb = stpool.tile([128, c.NCHK, HID], BF16)
            # message cols 64:128 of the table are never matmul'd but are
            # gathered; zero once so they are defined.
            nc.vector.memset(stage_sb[:, :, HID:], 0.0)

            agin_r = agin_d[:].rearrange("(k p) f -> p k f", p=128)
            oneh_r = oneh_d[:].rearrange("(k p) f -> p k f", p=128)

            nreg = nc.gpsimd.alloc_register("nidx")
            _regval = [None]

            def set_nreg(v):
                if _regval[0] != v:
                    nc.gpsimd.reg_mov(nreg, v)
                    _regval[0] = v

            BK = 8
            for l in range(3):
                # ---- transform: stage = dinv * (h @ W) ----
                for kk in range(cdiv(c.NCHK, BK)):
                    s = kk * BK
                    e = min(c.NCHK, s + BK)
                    if l == 0:
                        xt = xtpool.tile([c.IN, BK * 128], F32, tag="xt")
                        nc.sync.dma_start(
                            xt[:, : (e - s) * 128],
                            xT_d[:, s * 128:e * 128])
                    for k in range(s, e):
                        if l == 0:
                            lhsT = xt[:, (k - s) * 128:(k - s + 1) * 128]
                            Wl = W1_sb
                        else:
                            lhsT = hT_sb[:, k * 128:(k + 1) * 128]
                            Wl = W2_sb if l == 1 else W3_sb
                        ps = psA.tile([128, HID], F32, space="PSUM")
                        nc.tensor.matmul(ps[:], lhsT, Wl[:], start=True, stop=True)
                        nc.vector.tensor_scalar_mul(
                            stage_sb[:, k, :HID], ps[:], dinv_sb[:, k:k + 1])
                for kk in range(cdiv(c.NCHK, BK)):
                    s = kk * BK
                    e = min(c.NCHK, s + BK)
                    nc.sync.dma_start(agin_r[:, s:e, :], stage_sb[:, s:e, :])
                nc.gpsimd.collective_compute(
                    "AllGather", ALU.bypass,
                    replica_groups=rg, ins=[agin_d[:]], outs=[agout_d[:]])

                # ---- waves: gather + one-hot matmul aggregation ----
                for v, wm in enumerate(waves):
                    widx = widxpool.tile([128, IDXC], I16, tag="widx",
                                         name=f"widx_{l}_{v}")
                    col0 = wm["slot0"] // 16
                    ncol = wm["nslots"] // 16
                    nc.sync.dma_start(widx[:, :ncol], gidx_d[:, col0:col0 + ncol])
                    wdsel = dselpool.tile([128, DSELC], F32, tag="wdsel",
                                          name=f"wdsel_{l}_{v}")
                    nc.sync.dma_start(
                        wdsel[:, : wm["npairs"]],
                        dsel_d[:, wm["pair0"]:wm["pair0"] + wm["npairs"]])

                    # seed accumulators with self-loop term t' (all chunk
                    # accumulators of a wave share one PSUM bank tile)
                    ppw = psW.tile([128, c.W * HID], F32, space="PSUM",
                                   tag="pp", name=f"pp_{l}_{v}")
                    pp = {}
                    for i, cc in enumerate(wm["chunks"]):
                        pp[cc] = ppw[:, i * HID:(i + 1) * HID]
                        nc.tensor.matmul(
                            pp[cc], identb_sb[:], stage_sb[:, cc, :HID],
                            start=True, stop=False)

                    # gathers (one per bucket)
                    mtiles = {}
                    for ic in wm["calls"]:
                        call = calls[ic]
                        n = call["n"]
                        b = call["b"]
                        mt = msgpool.tile([128, MAXT, 128], BF16, tag="msg",
                                          name=f"msg_{l}_{ic}")
                        lo = (call["slot0"] - wm["slot0"]) // 16
                        set_nreg(n)
                        nc.gpsimd.dma_gather(
                            mt[:, : n // 128, :],
                            agout_d[b * c.SRCW:(b + 1) * c.SRCW, :],
                            widx[:, lo:lo + n // 16], n, nreg, 128,
                            queue_num=ic % 4)
                        mtiles[ic] = mt

                    # one-hot matmul accumulation
                    for j in range(wm["pair0"], wm["pair0"] + wm["npairs"]):
                        pr = pairs[j]
                        st = selpool.tile([128, 128], BF16, tag="sel",
                                          name=f"sel_{l}_{j}")
                        nc.vector.tensor_scalar(
                            st[:], iota_sb[:],
                            wdsel[:, j - wm["pair0"]:j - wm["pair0"] + 1],
                            None, ALU.is_equal)
                        nc.tensor.matmul(
                            pp[pr["cc"]], st[:],
                            mtiles[pr["call"]][:, pr["t"], :HID],
                            start=False, stop=pr["stop"])

                    # finalize chunks of this wave
                    for cc in wm["chunks"]:
                        vv = finpool.tile([128, HID], F32, tag="v")
                        nc.vector.tensor_scalar_mul(
                            vv[:], pp[cc], dinv_sb[:, cc:cc + 1])
                        if l < 2:
                            pt = psT.tile([HID, 128], F32, space="PSUM", tag="tp")
                            nc.tensor.transpose(pt[:], vv[:], ident[:])
                            nc.scalar.activation(
                                hT_sb[:, cc * 128:(cc + 1) * 128], pt[:],
                                AF.Relu, bias=bcols_sb[:, l:l + 1])
                        else:
                            vb = finpool.tile([128, HID], F32, tag="vb")
                            nc.vector.tensor_add(vb[:], vv[:], b3rep_sb[:])
                            nc.vector.tensor_relu(h3_sb[:, cc, :], vb[:])

            # ---- mean pool ----
            pq = psA.tile([128, HID], F32, space="PSUM", tag="pool", bufs=1)
            for kk in range(cdiv(c.NCHK, BK)):
                s = kk * BK
                e = min(c.NCHK, s + BK)
                oh = ohpool.tile([128, BK, 128], BF16, tag="oh")
                nc.sync.dma_start(oh[:, : e - s, :], oneh_r[:, s:e, :])
                for k in range(s, e):
                    nc.tensor.matmul(
                        pq[:], oh[:, k - s, :], h3_sb[:, k, :],
                        start=(k == 0), stop=(k == c.NCHK - 1))
            pl = res.tile([128, HID], F32)
            nc.vector.tensor_copy(pl[:], pq[:])
            nc.sync.dma_start(plin_d[:], pl[:])
            nc.gpsimd.collective_compute(
                "AllReduce", ALU.add,
                replica_groups=rg, ins=[plin_d[:]], outs=[plout_d[:]])
            plr = res.tile([128, HID], F32)
            nc.sync.dma_start(plr[:], plout_d[:])
            plm = res.tile([128, HID], F32)
            nc.vector.tensor_scalar_mul(plm[:], plr[:], cinv_sb[:])
            pst = psT.tile([HID, 128], F32, space="PSUM", tag="tp")
            nc.tensor.transpose(pst[:], plm[:], ident[:])
            plT = res.tile([HID, 128], F32)
            nc.vector.tensor_copy(plT[:], pst[:])
            psC = psT.tile([G, OUT], F32, space="PSUM", tag="pC", bufs=1)
            nc.tensor.matmul(psC[:], plT[:, :G], Wc_sb[:], start=True, stop=True)
            lg = res.tile([G, OUT], F32)
            nc.vector.tensor_add(lg[:], psC[:, :], bcrep_sb[:G, :])
            mx = res.tile([G, 1], F32)
            nc.vector.tensor_reduce(mx[:], lg[:], mybir.AxisListType.X, ALU.max)
            lgs = res.tile([G, OUT], F32)
            nc.vector.tensor_scalar_sub(lgs[:], lg[:], mx[:])
            ex = res.tile([G, OUT], F32)
            nc.scalar.activation(ex[:], lgs[:], AF.Exp)
            sm = res.tile([G, 1], F32)
            nc.vector.tensor_reduce(sm[:], ex[:], mybir.AxisListType.X, ALU.add)
            ls = res.tile([G, 1], F32)
            nc.scalar.activation(ls[:], sm[:], AF.Ln)
            yt = res.tile([G, OUT], F32)
            nc.vector.tensor_scalar_sub(yt[:], lgs[:], ls[:])
            nc.sync.dma_start(y_d[:], yt[:])

    return nc


def _finalize(nc):
    nc.compile()
    fix_multiwait(nc)


def run(inputs, cfg, profile_dir=None):
    from concourse.bass_utils import run_bass_kernel_spmd

    in_maps, meta = prep(inputs, cfg)
    nc = build(cfg, meta)
    _finalize(nc)
    if profile_dir is not None:
        from trn_agent_boot.trn_boot import _ntff_profile_via_ctypes
        hook = _ntff_profile_via_ctypes("/opt/axon/libaxon_pjrt.so")
        with hook(profile_dir, [0]):
            res = run_bass_kernel_spmd(nc, in_maps, core_ids=list(range(cfg.C)))
    else:
        res = run_bass_kernel_spmd(nc, in_maps, core_ids=list(range(cfg.C)))
    return res.results[0]["y"]


# ---------------------------------------------------------------------------
N_NODES, N_EDGES, IN_DIM, HID_DIM, N_GRAPHS, OUT_DIM = 100_000, 1_600_000, 128, 64, 128, 3


def kernel(**inputs):
    import os
    cfg = Cfg(N_NODES, N_EDGES, IN_DIM, HID_DIM, N_GRAPHS, OUT_DIM, W=7)
    out = run(inputs, cfg, profile_dir=os.environ.get("GNN_PROFILE_DIR"))
    return np.asarray(out, np.float32)


# revision 15
# speedup vs baseline: 2.0706x; 1.2007x over previous
"""3-layer GCN + mean-pool + classifier for Trainium2, SPMD on 8 NeuronCores.

Self-contained: kernel(**inputs) takes the full-size numpy inputs, does the
host-side graph partitioning, builds/compiles a Bass/Tile kernel, runs it on
cores 0-7 via run_bass_kernel_spmd, and returns the [128, 3] log-softmax
output.

Distribution: nodes are dst-sharded across the 8 cores. Per GCN layer each
core computes t' = dinv * (h @ W) for its shard (TensorE), casts to bf16 and
AllGathers the full node table, then per-edge messages are dma_gathered from
the HBM table (256 B rows). Aggregation runs entirely on TensorE: edges are
grouped into waves of 7 dst chunks (128 nodes each); per 128-edge message
tile the DVE builds a one-hot selection matrix S^T[e,d] = (dst_local[e]==d)
by iota-compare, and matmul(psum[cc], S^T, msgs) accumulates the scatter-add
in PSUM. No dma_scatter_add and no duplicate-dst rounds are needed (duplicate
dst within a tile just means two one-hot rows share a column). Gather calls
obey the HW limits (idx value < 8192 via 16 source buckets, <= 1024 idxs per
call); the per-(wave,bucket) call sizes are max-padded across cores so all 8
cores run an identical program (pads gather row 0 and select no column).
The GCN normalization deg^-1/2 (A+I) deg^-1/2 factorizes into a pre-scale of
t' and a post-scale of the aggregate; self-loops become an identity-matmul
seed of the PSUM accumulator. Mean-pooling is a one-hot matmul with an
AllReduce of per-core partials; the classifier + log_softmax run replicated.
"""
import sys

sys.path.insert(0, "/opt/trn_rl_repo")

import numpy as np
import ml_dtypes
import concourse.bacc as bacc
import concourse.mybir as mybir
import concourse.tile as tile

# ---------------------------------------------------------------------------
# Workarounds: this walrus build rejects >1 sync-wait per instruction.

import concourse.tile as _tile
import concourse.mybir as _mybir
from concourse.vector_clock import ScopedClock as _ScopedClock


def _split_waits_tail(nc, inst):
    si = inst.ins.sync_info
    if si is None or not si.on_wait or len(si.on_wait) <= 1:
        return
    waits = list(si.on_wait)
    inst.ins.sync_info = _mybir.SyncInfo(on_wait=[], on_update=list(si.on_update or []))
    for w in waits:
        nop = nc.sync.nop()
        nop.ins.sync_info = _mybir.SyncInfo(on_wait=[w], on_update=[])


def _drain_and_barrier(self, tick_clock, wait_clock):
    nc = self.nc
    probe = nc.sync.nop()
    wait_clock.add_sem_waits(probe.ins, _ScopedClock({None: tick_clock.global_clock}))
    _split_waits_tail(nc, probe)
    nc.sync.drain()
    nc.all_engine_barrier()
    assert self.sems is not None
    popped = nc._tile_sem_poison_stack.pop()
    assert popped is self._sem_poison
    nc.clear_and_free_semaphores(list(self.sems.allocated().values()))
    nc.all_engine_barrier()


_tile.TileContext._drain_and_barrier = _drain_and_barrier


def fix_multiwait(nc):
    """Rewrite every >1-wait instruction into wait-nops + 1-wait instruction."""
    for f in nc.m.functions:
        for blk in f.blocks:
            insts = blk.instructions            # live list (rust-backed)
            i = 0
            while i < len(insts):
                inst = insts[i]
                si = inst.sync_info
                if si is not None and si.on_wait and len(si.on_wait) > 1:
                    waits = list(si.on_wait)
                    eng = inst.engine
                    inst.sync_info = _mybir.SyncInfo(
                        on_wait=[waits[-1]], on_update=list(si.on_update or [])
                    )
                    for j, w in enumerate(waits[:-1]):
                        nop = nc.engines[eng].nop(hint="mwfix")
                        popped = False
                        for f2 in nc.m.functions:
                            for b2 in f2.blocks:
                                l2 = b2.instructions
                                if l2 and l2[-1].name == nop.ins.name:
                                    l2.pop()
                                    popped = True
                                    break
                            if popped:
                                break
                        assert popped, "could not relocate mwfix nop"
                        nop.ins.sync_info = _mybir.SyncInfo(on_wait=[w], on_update=[])
                        insts.insert(i + j, nop.ins)
                    i += len(waits) - 1
                i += 1


# ---------------------------------------------------------------------------

F32 = mybir.dt.float32
BF16 = mybir.dt.bfloat16
I16 = mybir.dt.int16
AF = mybir.ActivationFunctionType
ALU = mybir.AluOpType
NPBF = ml_dtypes.bfloat16


def cdiv(a, b):
    return (a + b - 1) // b


def rup(a, b):
    return cdiv(a, b) * b


class Cfg:
    def __init__(self, N, E, IN, HID, G, OUT, W=7):
        self.C = 8
        self.N, self.E, self.IN, self.HID, self.G, self.OUT = N, E, IN, HID, G, OUT
        assert N % self.C == 0
        self.NSH = N // self.C                  # 12500 nodes per core
        self.TROW = rup(self.NSH, 128)          # 12544
        self.NCHK = self.TROW // 128            # 98 dst chunks
        self.NBUK = 16
        assert (self.C * self.TROW) % self.NBUK == 0
        self.SRCW = self.C * self.TROW // self.NBUK   # 6272 rows per bucket
        assert self.SRCW <= 8191  # HW: gather idx value must fit 13 bits
        self.W = W                              # dst chunks per wave
        self.NW = cdiv(self.NCHK, W)
        self.MAXCALL = 1024                     # HW: gather <= 1024 idxs
        assert G <= 128


def _wrap_cols(a):
    """[n slots] (n % 16 == 0) -> wrapped [128, n // 16] int16."""
    w = a.reshape(-1, 16).T  # [16, n//16]
    return np.tile(w, (8, 1)).astype(np.int16)


def prep(inputs, cfg):
    c = cfg
    x = np.asarray(inputs["x"], np.float32)
    ei = np.asarray(inputs["edge_index"], np.int64)
    batch = np.asarray(inputs["batch"], np.int64)
    W1 = np.asarray(inputs["W1"], np.float32); b1 = np.asarray(inputs["b1"], np.float32)
    W2 = np.asarray(inputs["W2"], np.float32); b2 = np.asarray(inputs["b2"], np.float32)
    W3 = np.asarray(inputs["W3"], np.float32); b3 = np.asarray(inputs["b3"], np.float32)
    Wc = np.asarray(inputs["Wc"], np.float32); bc = np.asarray(inputs["bc"], np.float32)

    src = ei[0].astype(np.int64)
    dst = ei[1].astype(np.int64)
    deg = np.bincount(dst, minlength=c.N).astype(np.float32) + 1.0
    dinv = 1.0 / np.sqrt(deg)

    HID = c.HID
    W3p = np.zeros((HID, HID), np.float32); W3p[:, : W3.shape[1]] = W3
    b3p = np.zeros((HID,), np.float32); b3p[: b3.shape[0]] = b3
    Wcp = np.zeros((HID, c.OUT), np.float32); Wcp[: Wc.shape[0]] = Wc

    core_of = src // c.NSH
    trow_src = core_of * c.TROW + (src - core_of * c.NSH)
    buk = trow_src // c.SRCW
    gix = trow_src - buk * c.SRCW
    dcore = dst // c.NSH
    dloc = dst - dcore * c.NSH
    chk = dloc >> 7
    din = dloc & 127
    wav = chk // c.W

    NW, NB = c.NW, c.NBUK
    # --- SPMD-identical call structure: max segment size across cores
    SEG = np.zeros((NW, NB), np.int64)
    pc_data = []
    for ci in range(c.C):
        sel = np.flatnonzero(dcore == ci)
        o = sel[np.lexsort((din[sel], chk[sel], buk[sel], wav[sel]))]
        key = wav[o] * NB + buk[o]
        cnt = np.bincount(key, minlength=NW * NB)
        starts = np.concatenate([[0], np.cumsum(cnt)])
        pc_data.append((o, starts))
        SEG = np.maximum(SEG, cnt.reshape(NW, NB))
    SEGP = np.maximum(rup(SEG, 128), 128)
    assert SEGP.max() <= c.MAXCALL, f"segment overflow: {SEGP.max()}"

    calls = []      # dict(v, b, n, slot0)
    waves = []      # dict(chunks, calls, slot0, nslots, pair0, npairs)
    slot0 = 0
    for v in range(NW):
        wcalls = []
        ws0 = slot0
        for b in range(NB):
            n = int(SEGP[v, b])
            calls.append(dict(v=v, b=b, n=n, slot0=slot0))
            wcalls.append(len(calls) - 1)
            slot0 += n
        waves.append(dict(
            chunks=list(range(v * c.W, min((v + 1) * c.W, c.NCHK))),
            calls=wcalls, slot0=ws0, nslots=slot0 - ws0))
    TOT = slot0

    # --- per-core slot fill
    gs = np.zeros((c.C, TOT), np.int64)
    sch = np.full((c.C, TOT), -1, np.int64)
    sdi = np.zeros((c.C, TOT), np.int64)
    for ci in range(c.C):
        o, starts = pc_data[ci]
        for call in calls:
            k = call["v"] * NB + call["b"]
            e0, e1 = int(starts[k]), int(starts[k + 1])
            n = e1 - e0
            s0 = call["slot0"]
            idxs = o[e0:e1]
            gs[ci, s0:s0 + n] = gix[idxs]
            sch[ci, s0:s0 + n] = chk[idxs]
            sdi[ci, s0:s0 + n] = din[idxs]

    # --- pairs: per 128-slot tile, union of chunks present (across cores)
    pairs = []      # dict(call, t, cc, stop)
    for v, wm in enumerate(waves):
        p0 = len(pairs)
        for ic in wm["calls"]:
            call = calls[ic]
            for t in range(call["n"] // 128):
                sl = slice(call["slot0"] + t * 128, call["slot0"] + (t + 1) * 128)
                u = np.unique(sch[:, sl])
                u = u[u >= 0]
                for cc in u:
                    pairs.append(dict(call=ic, t=t, cc=int(cc), stop=False))
        last = {}
        for j in range(p0, len(pairs)):
            last[pairs[j]["cc"]] = j
        for j in range(p0, len(pairs)):
            pairs[j]["stop"] = last[pairs[j]["cc"]] == j
        for cc in wm["chunks"]:
            assert cc in last, f"chunk {cc} has no edges in wave {v}"
        wm["pair0"] = p0
        wm["npairs"] = len(pairs) - p0
    NPAIRS = len(pairs)

    # --- per-core one-hot select columns
    dsel = np.full((c.C, 128, NPAIRS), 200.0, np.float32)
    for j, pr in enumerate(pairs):
        call = calls[pr["call"]]
        sl = slice(call["slot0"] + pr["t"] * 128, call["slot0"] + (pr["t"] + 1) * 128)
        m = sch[:, sl] == pr["cc"]
        dsel[:, :, j] = np.where(m, sdi[:, sl], 200).astype(np.float32)

    cnt = np.bincount(batch, minlength=c.G).astype(np.float32)
    cntinv = (1.0 / np.maximum(cnt, 1.0)).astype(np.float32)

    iota_row = np.tile(np.arange(128, dtype=np.float32), (128, 1)).astype(NPBF)
    identb = np.eye(128, dtype=np.float32).astype(NPBF)

    in_maps = []
    for ci in range(c.C):
        lo, hi = ci * c.NSH, (ci + 1) * c.NSH
        xT = np.zeros((c.IN, c.TROW), np.float32)
        xT[:, : c.NSH] = x[lo:hi].T
        dv = np.zeros((c.TROW,), np.float32)
        dv[: c.NSH] = dinv[lo:hi]
        dinv2d = dv.reshape(c.NCHK, 128).T.copy()

        oneh = np.zeros((c.TROW, 128), np.float32)
        oneh[np.arange(c.NSH), batch[lo:hi]] = 1.0

        bcols = np.stack([b1, b2, b3p], axis=1)
        b3rep = np.tile(b3p[None, :], (128, 1))
        bcrep = np.tile(bc[None, :], (128, 1))
        cinv = np.zeros((128, 1), np.float32)
        cinv[: c.G, 0] = cntinv

        in_maps.append(dict(
            xT=xT, dinv2d=dinv2d,
            gidx=_wrap_cols(gs[ci]),
            dsel=dsel[ci],
            oneh=oneh.astype(NPBF),
            W1d=W1, W2d=W2.astype(NPBF), W3d=W3p.astype(NPBF),
            bcols=bcols, b3rep=b3rep,
            Wcp=Wcp, bcrep=bcrep, cinv=cinv,
            iota=iota_row, identb=identb,
        ))

    meta = dict(calls=calls, waves=waves, pairs=pairs, TOT=TOT, NPAIRS=NPAIRS)
    return in_maps, meta


def build(cfg, meta):
    c = cfg
    HID, G, OUT = c.HID, c.G, c.OUT
    calls, waves, pairs = meta["calls"], meta["waves"], meta["pairs"]
    TOT, NPAIRS = meta["TOT"], meta["NPAIRS"]
    IDXC = max(wm["nslots"] // 16 for wm in waves)
    DSELC = max(wm["npairs"] for wm in waves)
    MAXT = c.MAXCALL // 128     # 8 tiles per call

    nc = bacc.Bacc("TRN2", num_devices=c.C, dynamic_dma_scratch_size=65536,
                   num_swdge_queues=4)

    def ein(name, shape, dt=F32):
        return nc.dram_tensor(name, shape, dt, kind="ExternalInput")

    xT_d = ein("xT", [c.IN, c.TROW])
    dinv_d = ein("dinv2d", [128, c.NCHK])
    gidx_d = ein("gidx", [128, TOT // 16], I16)
    dsel_d = ein("dsel", [128, NPAIRS])
    oneh_d = ein("oneh", [c.TROW, 128], BF16)
    W1_d = ein("W1d", [c.IN, HID])
    W2_d = ein("W2d", [HID, HID], BF16)
    W3_d = ein("W3d", [HID, HID], BF16)
    bcols_d = ein("bcols", [HID, 3])
    b3rep_d = ein("b3rep", [128, HID])
    Wc_d = ein("Wcp", [HID, OUT])
    bcrep_d = ein("bcrep", [128, OUT])
    cinv_d = ein("cinv", [128, 1])
    iota_d = ein("iota", [128, 128], BF16)
    identb_d = ein("identb", [128, 128], BF16)

    agin_d = nc.dram_tensor("agin", [c.TROW, 128], BF16, kind="Internal")
    agout_d = nc.dram_tensor(
        "agout", [c.C * c.TROW, 128], BF16, kind="Internal", addr_space="Shared")
    plin_d = nc.dram_tensor("plin", [128, HID], F32, kind="Internal")
    plout_d = nc.dram_tensor(
        "plout", [128, HID], F32, kind="Internal", addr_space="Shared")
    y_d = nc.dram_tensor("y", [G, OUT], F32, kind="ExternalOutput")

    rg = [list(range(c.C))]

    with tile.TileContext(nc) as tc:
        with (
            tc.tile_pool(name="res", bufs=1) as res,
            tc.tile_pool(name="stage", bufs=1) as stpool,
            tc.tile_pool(name="xt", bufs=3) as xtpool,
            tc.tile_pool(name="msg", bufs=28) as msgpool,
            tc.tile_pool(name="widx", bufs=2) as widxpool,
            tc.tile_pool(name="wdsel", bufs=2) as dselpool,
            tc.tile_pool(name="sel", bufs=6) as selpool,
            tc.tile_pool(name="fin", bufs=4) as finpool,
            tc.tile_pool(name="oh", bufs=2) as ohpool,
            tc.tile_pool(name="psA", bufs=2, space="PSUM") as psA,
            tc.tile_pool(name="psT", bufs=2, space="PSUM") as psT,
            tc.tile_pool(name="psW", bufs=2, space="PSUM") as psW,
        ):
            from concourse.masks import make_identity
            ident = res.tile([128, 128], F32)
            make_identity(nc, ident[:])
            identb_sb = res.tile([128, 128], BF16)
            nc.sync.dma_start(identb_sb[:], identb_d[:])
            iota_sb = res.tile([128, 128], BF16)
            nc.sync.dma_start(iota_sb[:], iota_d[:])
            dinv_sb = res.tile([128, c.NCHK], F32)
            nc.sync.dma_start(dinv_sb[:], dinv_d[:])
            W1_sb = res.tile([c.IN, HID], F32, name="w1")
            nc.sync.dma_start(W1_sb[:], W1_d[:])
            W2_sb = res.tile([HID, HID], BF16, name="w2")
            nc.sync.dma_start(W2_sb[:], W2_d[:])
            W3_sb = res.tile([HID, HID], BF16, name="w3")
            nc.sync.dma_start(W3_sb[:], W3_d[:])
            bcols_sb = res.tile([HID, 3], F32)
            nc.sync.dma_start(bcols_sb[:], bcols_d[:])
            b3rep_sb = res.tile([128, HID], F32)
            nc.sync.dma_start(b3rep_sb[:], b3rep_d[:])
            Wc_sb = res.tile([HID, OUT], F32)
            nc.sync.dma_start(Wc_sb[:], Wc_d[:])
            bcrep_sb = res.tile([128, OUT], F32)
            nc.sync.dma_start(bcrep_sb[:], bcrep_d[:])
            cinv_sb = res.tile([128, 1], F32)
            nc.sync.dma_start(cinv_sb[:], cinv_d[:])

            stage_sb = stpool.tile([128, c.NCHK, 128], BF16)
            hT_sb = stpool.tile([HID, c.TROW], BF16)
            h3_sb = stpool.tile([128, c.NCHK, HID], BF16)
            # message cols 64:128 of the table are never matmul'd but are
            # gathered; zero once so they are defined.
            nc.vector.memset(stage_sb[:, :, HID:], 0.0)

            agin_r = agin_d[:].rearrange("(k p) f -> p k f", p=128)
            oneh_r = oneh_d[:].rearrange("(k p) f -> p k f", p=128)

            nreg = nc.gpsimd.alloc_register("nidx")
            _regval = [None]

            def set_nreg(v):
                if _regval[0] != v:
                    nc.gpsimd.reg_mov(nreg, v)
                    _regval[0] = v

            BK = 8
            for l in range(3):
                # ---- transform: stage = dinv * (h @ W) ----
                for kk in range(cdiv(c.NCHK, BK)):
                    s = kk * BK
                    e = min(c.NCHK, s + BK)
                    if l == 0:
                        xt = xtpool.tile([c.IN, BK * 128], F32, tag="xt")
                        nc.sync.dma_start(
                            xt[:, : (e - s) * 128],
                            xT_d[:, s * 128:e * 128])
                    for k in range(s, e):
                        if l == 0:
                            lhsT = xt[:, (k - s) * 128:(k - s + 1) * 128]
                            Wl = W1_sb
                        else:
                            lhsT = hT_sb[:, k * 128:(k + 1) * 128]
                            Wl = W2_sb if l == 1 else W3_sb
                        ps = psA.tile([128, HID], F32, space="PSUM")
                        nc.tensor.matmul(ps[:], lhsT, Wl[:], start=True, stop=True)
                        nc.vector.tensor_scalar_mul(
                            stage_sb[:, k, :HID], ps[:], dinv_sb[:, k:k + 1])
                for kk in range(cdiv(c.NCHK, BK)):
                    s = kk * BK
                    e = min(c.NCHK, s + BK)
                    nc.sync.dma_start(agin_r[:, s:e, :], stage_sb[:, s:e, :])
                nc.gpsimd.collective_compute(
                    "AllGather", ALU.bypass,
                    replica_groups=rg, ins=[agin_d[:]], outs=[agout_d[:]])

                # ---- waves: gather + one-hot matmul aggregation ----
                for v, wm in enumerate(waves):
                    widx = widxpool.tile([128, IDXC], I16, tag="widx",
                                         name=f"widx_{l}_{v}")
                    col0 = wm["slot0"] // 16
                    ncol = wm["nslots"] // 16
                    nc.sync.dma_start(widx[:, :ncol], gidx_d[:, col0:col0 + ncol])
                    wdsel = dselpool.tile([128, DSELC], F32, tag="wdsel",
                                          name=f"wdsel_{l}_{v}")
                    nc.sync.dma_start(
                        wdsel[:, : wm["npairs"]],
                        dsel_d[:, wm["pair0"]:wm["pair0"] + wm["npairs"]])

                    # seed accumulators with self-loop term t' (all chunk
                    # accumulators of a wave share one PSUM bank tile)
                    ppw = psW.tile([128, c.W * HID], F32, space="PSUM",
                                   tag="pp", name=f"pp_{l}_{v}")
                    pp = {}
                    for i, cc in enumerate(wm["chunks"]):
                        pp[cc] = ppw[:, i * HID:(i + 1) * HID]
                        nc.tensor.matmul(
                            pp[cc], identb_sb[:], stage_sb[:, cc, :HID],
                            start=True, stop=False)

                    # gathers (one per bucket)
                    mtiles = {}
                    for ic in wm["calls"]:
                        call = calls[ic]
                        n = call["n"]
                        b = call["b"]
                        mt = msgpool.tile([128, MAXT, 128], BF16, tag="msg",
                                          name=f"msg_{l}_{ic}")
                        lo = (call["slot0"] - wm["slot0"]) // 16
                        set_nreg(n)
                        nc.gpsimd.dma_gather(
                            mt[:, : n // 128, :],
                            agout_d[b * c.SRCW:(b + 1) * c.SRCW, :],
                            widx[:, lo:lo + n // 16], n, nreg, 128,
                            queue_num=ic % 4, single_packet=False)
                        mtiles[ic] = mt

                    # one-hot matmul accumulation
                    for j in range(wm["pair0"], wm["pair0"] + wm["npairs"]):
                        pr = pairs[j]
                        st = selpool.tile([128, 128], BF16, tag="sel",
                                          name=f"sel_{l}_{j}")
                        nc.vector.tensor_scalar(
                            st[:], iota_sb[:],
                            wdsel[:, j - wm["pair0"]:j - wm["pair0"] + 1],
                            None, ALU.is_equal)
                        nc.tensor.matmul(
                            pp[pr["cc"]], st[:],
                            mtiles[pr["call"]][:, pr["t"], :HID],
                            start=False, stop=pr["stop"])

                    # finalize chunks of this wave
                    for cc in wm["chunks"]:
                        vv = finpool.tile([128, HID], F32, tag="v")
                        nc.vector.tensor_scalar_mul(
                            vv[:], pp[cc], dinv_sb[:, cc:cc + 1])
                        if l < 2:
                            pt = psT.tile([HID, 128], F32, space="PSUM", tag="tp")
                            nc.tensor.transpose(pt[:], vv[:], ident[:])
                            nc.scalar.activation(
                                hT_sb[:, cc * 128:(cc + 1) * 128], pt[:],
                                AF.Relu, bias=bcols_sb[:, l:l + 1])
                        else:
                            vb = finpool.tile([128, HID], F32, tag="vb")
                            nc.vector.tensor_add(vb[:], vv[:], b3rep_sb[:])
                            nc.vector.tensor_relu(h3_sb[:, cc, :], vb[:])

            # ---- mean pool ----
            pq = psA.tile([128, HID], F32, space="PSUM", tag="pool", bufs=1)
            for kk in range(cdiv(c.NCHK, BK)):
                s = kk * BK
                e = min(c.NCHK, s + BK)
                oh = ohpool.tile([128, BK, 128], BF16, tag="oh")
                nc.sync.dma_start(oh[:, : e - s, :], oneh_r[:, s:e, :])
                for k in range(s, e):
                    nc.tensor.matmul(
                        pq[:], oh[:, k - s, :], h3_sb[:, k, :],
                        start=(k == 0), stop=(k == c.NCHK - 1))
            pl = res.tile([128, HID], F32)
            nc.vector.tensor_copy(pl[:], pq[:])
            nc.sync.dma_start(plin_d[:], pl[:])
            nc.gpsimd.collective_compute(
                "AllReduce", ALU.add,
                replica_groups=rg, ins=[plin_d[:]], outs=[plout_d[:]])
            plr = res.tile([128, HID], F32)
            nc.sync.dma_start(plr[:], plout_d[:])
            plm = res.tile([128, HID], F32)
            nc.vector.tensor_scalar_mul(plm[:], plr[:], cinv_sb[:])
            pst = psT.tile([HID, 128], F32, space="PSUM", tag="tp")
            nc.tensor.transpose(pst[:], plm[:], ident[:])
            plT = res.tile([HID, 128], F32)
            nc.vector.tensor_copy(plT[:], pst[:])
            psC = psT.tile([G, OUT], F32, space="PSUM", tag="pC", bufs=1)
            nc.tensor.matmul(psC[:], plT[:, :G], Wc_sb[:], start=True, stop=True)
            lg = res.tile([G, OUT], F32)
            nc.vector.tensor_add(lg[:], psC[:, :], bcrep_sb[:G, :])
            mx = res.tile([G, 1], F32)
            nc.vector.tensor_reduce(mx[:], lg[:], mybir.AxisListType.X, ALU.max)
            lgs = res.tile([G, OUT], F32)
            nc.vector.tensor_scalar_sub(lgs[:], lg[:], mx[:])
            ex = res.tile([G, OUT], F32)
            nc.scalar.activation(ex[:], lgs[:], AF.Exp)
            sm = res.tile([G, 1], F32)
            nc.vector.tensor_reduce(sm[:], ex[:], mybir.AxisListType.X, ALU.add)
            ls = res.tile([G, 1], F32)
            nc.scalar.activation(ls[:], sm[:], AF.Ln)
            yt = res.tile([G, OUT], F32)
            nc.vector.tensor_scalar_sub(yt[:], lgs[:], ls[:])
            nc.sync.dma_start(y_d[:], yt[:])

    return nc


def _finalize(nc):
    nc.compile()
    fix_multiwait(nc)


def run(inputs, cfg, profile_dir=None):
    from concourse.bass_utils import run_bass_kernel_spmd

    in_maps, meta = prep(inputs, cfg)
    nc = build(cfg, meta)
    _finalize(nc)
    if profile_dir is not None:
        from trn_agent_boot.trn_boot import _ntff_profile_via_ctypes
        hook = _ntff_profile_via_ctypes("/opt/axon/libaxon_pjrt.so")
        with hook(profile_dir, [0]):
            res = run_bass_kernel_spmd(nc, in_maps, core_ids=list(range(cfg.C)))
    else:
        res = run_bass_kernel_spmd(nc, in_maps, core_ids=list(range(cfg.C)))
    return res.results[0]["y"]


# ---------------------------------------------------------------------------
N_NODES, N_EDGES, IN_DIM, HID_DIM, N_GRAPHS, OUT_DIM = 100_000, 1_600_000, 128, 64, 128, 3


def kernel(**inputs):
    import os
    cfg = Cfg(N_NODES, N_EDGES, IN_DIM, HID_DIM, N_GRAPHS, OUT_DIM, W=7)
    out = run(inputs, cfg, profile_dir=os.environ.get("GNN_PROFILE_DIR"))
    return np.asarray(out, np.float32)


# revision 27
# speedup vs baseline: 2.3868x; 1.1527x over previous
"""3-layer GCN + mean-pool + classifier for Trainium2, SPMD on 8 NeuronCores.

Self-contained: kernel(**inputs) takes the full-size numpy inputs, does the
host-side graph partitioning, builds/compiles a Bass/Tile kernel, runs it on
cores 0-7 via run_bass_kernel_spmd, and returns the [128, 3] log-softmax
output.

Distribution: nodes are dst-sharded across the 8 cores. Per GCN layer each
core computes t' = dinv * (h @ W) for its shard (TensorE), casts to bf16 and
AllGathers the full node table, then per-edge messages are dma_gathered from
the HBM table (256 B rows). Aggregation runs entirely on TensorE: edges are
grouped into waves of 7 dst chunks (128 nodes each); per 128-edge message
tile the DVE builds a one-hot selection matrix S^T[e,d] = (dst_local[e]==d)
by iota-compare, and matmul(psum[cc], S^T, msgs) accumulates the scatter-add
in PSUM. No dma_scatter_add and no duplicate-dst rounds are needed (duplicate
dst within a tile just means two one-hot rows share a column). Gather calls
obey the HW limits (idx value < 8192 via 16 source buckets, <= 1024 idxs per
call); the per-(wave,bucket) call sizes are max-padded across cores so all 8
cores run an identical program (pads gather row 0 and select no column).
The GCN normalization deg^-1/2 (A+I) deg^-1/2 factorizes into a pre-scale of
t' and a post-scale of the aggregate; self-loops become an identity-matmul
seed of the PSUM accumulator. Mean-pooling is a one-hot matmul with an
AllReduce of per-core partials; the classifier + log_softmax run replicated.
"""
import sys

sys.path.insert(0, "/opt/trn_rl_repo")

import numpy as np
import ml_dtypes
import concourse.bacc as bacc
import concourse.mybir as mybir
import concourse.tile as tile

# ---------------------------------------------------------------------------
# Workarounds: this walrus build rejects >1 sync-wait per instruction.

import concourse.tile as _tile
import concourse.mybir as _mybir
from concourse.vector_clock import ScopedClock as _ScopedClock


def _split_waits_tail(nc, inst):
    si = inst.ins.sync_info
    if si is None or not si.on_wait or len(si.on_wait) <= 1:
        return
    waits = list(si.on_wait)
    inst.ins.sync_info = _mybir.SyncInfo(on_wait=[], on_update=list(si.on_update or []))
    for w in waits:
        nop = nc.sync.nop()
        nop.ins.sync_info = _mybir.SyncInfo(on_wait=[w], on_update=[])


def _drain_and_barrier(self, tick_clock, wait_clock):
    nc = self.nc
    probe = nc.sync.nop()
    wait_clock.add_sem_waits(probe.ins, _ScopedClock({None: tick_clock.global_clock}))
    _split_waits_tail(nc, probe)
    nc.sync.drain()
    nc.all_engine_barrier()
    assert self.sems is not None
    popped = nc._tile_sem_poison_stack.pop()
    assert popped is self._sem_poison
    nc.clear_and_free_semaphores(list(self.sems.allocated().values()))
    nc.all_engine_barrier()


_tile.TileContext._drain_and_barrier = _drain_and_barrier


def fix_multiwait(nc):
    """Rewrite every >1-wait instruction into wait-nops + 1-wait instruction."""
    for f in nc.m.functions:
        for blk in f.blocks:
            insts = blk.instructions            # live list (rust-backed)
            i = 0
            while i < len(insts):
                inst = insts[i]
                si = inst.sync_info
                if si is not None and si.on_wait and len(si.on_wait) > 1:
                    waits = list(si.on_wait)
                    eng = inst.engine
                    inst.sync_info = _mybir.SyncInfo(
                        on_wait=[waits[-1]], on_update=list(si.on_update or [])
                    )
                    for j, w in enumerate(waits[:-1]):
                        nop = nc.engines[eng].nop(hint="mwfix")
                        popped = False
                        for f2 in nc.m.functions:
                            for b2 in f2.blocks:
                                l2 = b2.instructions
                                if l2 and l2[-1].name == nop.ins.name:
                                    l2.pop()
                                    popped = True
                                    break
                            if popped:
                                break
                        assert popped, "could not relocate mwfix nop"
                        nop.ins.sync_info = _mybir.SyncInfo(on_wait=[w], on_update=[])
                        insts.insert(i + j, nop.ins)
                    i += len(waits) - 1
                i += 1


# ---------------------------------------------------------------------------

F32 = mybir.dt.float32
BF16 = mybir.dt.bfloat16
I16 = mybir.dt.int16
AF = mybir.ActivationFunctionType
ALU = mybir.AluOpType
NPBF = ml_dtypes.bfloat16


def cdiv(a, b):
    return (a + b - 1) // b


def rup(a, b):
    return cdiv(a, b) * b


class Cfg:
    def __init__(self, N, E, IN, HID, G, OUT, W=7):
        self.C = 8
        self.N, self.E, self.IN, self.HID, self.G, self.OUT = N, E, IN, HID, G, OUT
        assert N % self.C == 0
        self.NSH = N // self.C                  # 12500 nodes per core
        self.TROW = rup(self.NSH, 128)          # 12544
        self.NCHK = self.TROW // 128            # 98 dst chunks
        self.NBUK = 16
        assert (self.C * self.TROW) % self.NBUK == 0
        self.SRCW = self.C * self.TROW // self.NBUK   # 6272 rows per bucket
        assert self.SRCW <= 8191  # HW: gather idx value must fit 13 bits
        self.W = W                              # dst chunks per wave
        self.NW = cdiv(self.NCHK, W)
        self.MAXCALL = 1024                     # HW: gather <= 1024 idxs
        assert G <= 128


def _wrap_cols(a):
    """[n slots] (n % 16 == 0) -> wrapped [128, n // 16] int16."""
    w = a.reshape(-1, 16).T  # [16, n//16]
    return np.tile(w, (8, 1)).astype(np.int16)


def prep(inputs, cfg):
    c = cfg
    x = np.asarray(inputs["x"], np.float32)
    ei = np.asarray(inputs["edge_index"], np.int64)
    batch = np.asarray(inputs["batch"], np.int64)
    W1 = np.asarray(inputs["W1"], np.float32); b1 = np.asarray(inputs["b1"], np.float32)
    W2 = np.asarray(inputs["W2"], np.float32); b2 = np.asarray(inputs["b2"], np.float32)
    W3 = np.asarray(inputs["W3"], np.float32); b3 = np.asarray(inputs["b3"], np.float32)
    Wc = np.asarray(inputs["Wc"], np.float32); bc = np.asarray(inputs["bc"], np.float32)

    src = ei[0].astype(np.int64)
    dst = ei[1].astype(np.int64)
    deg = np.bincount(dst, minlength=c.N).astype(np.float32) + 1.0
    dinv = 1.0 / np.sqrt(deg)

    HID = c.HID
    W3p = np.zeros((HID, HID), np.float32); W3p[:, : W3.shape[1]] = W3
    b3p = np.zeros((HID,), np.float32); b3p[: b3.shape[0]] = b3
    Wcp = np.zeros((HID, c.OUT), np.float32); Wcp[: Wc.shape[0]] = Wc

    core_of = src // c.NSH
    trow_src = core_of * c.TROW + (src - core_of * c.NSH)
    buk = trow_src // c.SRCW
    gix = trow_src - buk * c.SRCW
    dcore = dst // c.NSH
    dloc = dst - dcore * c.NSH
    chk = dloc >> 7
    din = dloc & 127
    wav = chk // c.W

    NW, NB = c.NW, c.NBUK
    # --- SPMD-identical call structure: max segment size across cores
    SEG = np.zeros((NW, NB), np.int64)
    pc_data = []
    for ci in range(c.C):
        sel = np.flatnonzero(dcore == ci)
        o = sel[np.lexsort((din[sel], chk[sel], buk[sel], wav[sel]))]
        key = wav[o] * NB + buk[o]
        cnt = np.bincount(key, minlength=NW * NB)
        starts = np.concatenate([[0], np.cumsum(cnt)])
        pc_data.append((o, starts))
        SEG = np.maximum(SEG, cnt.reshape(NW, NB))
    SEGP = np.maximum(rup(SEG, 128), 128)
    assert SEGP.max() <= c.MAXCALL, f"segment overflow: {SEGP.max()}"

    calls = []      # dict(v, b, n, slot0)
    waves = []      # dict(chunks, calls, slot0, nslots, pair0, npairs)
    slot0 = 0
    for v in range(NW):
        wcalls = []
        ws0 = slot0
        for b in range(NB):
            n = int(SEGP[v, b])
            calls.append(dict(v=v, b=b, n=n, slot0=slot0))
            wcalls.append(len(calls) - 1)
            slot0 += n
        waves.append(dict(
            chunks=list(range(v * c.W, min((v + 1) * c.W, c.NCHK))),
            calls=wcalls, slot0=ws0, nslots=slot0 - ws0))
    TOT = slot0

    # --- per-core slot fill
    gs = np.zeros((c.C, TOT), np.int64)
    sch = np.full((c.C, TOT), -1, np.int64)
    sdi = np.zeros((c.C, TOT), np.int64)
    for ci in range(c.C):
        o, starts = pc_data[ci]
        for call in calls:
            k = call["v"] * NB + call["b"]
            e0, e1 = int(starts[k]), int(starts[k + 1])
            n = e1 - e0
            s0 = call["slot0"]
            idxs = o[e0:e1]
            gs[ci, s0:s0 + n] = gix[idxs]
            sch[ci, s0:s0 + n] = chk[idxs]
            sdi[ci, s0:s0 + n] = din[idxs]

    # --- tiles: per 128-slot tile, union of chunks present (across cores).
    # One wide DVE one-hot is built per tile (out col = j*128 + din for the
    # tile's j-th chunk); each (tile, chunk) pair is one accumulate matmul.
    tiles = []      # dict(call, t, ccs)
    pairs = []      # dict(tile, j, cc, stop)
    for v, wm in enumerate(waves):
        t0 = len(tiles)
        p0 = len(pairs)
        for ic in wm["calls"]:
            call = calls[ic]
            for t in range(call["n"] // 128):
                sl = slice(call["slot0"] + t * 128, call["slot0"] + (t + 1) * 128)
                u = np.unique(sch[:, sl])
                u = [int(cc) for cc in u if cc >= 0]
                ti = len(tiles)
                tiles.append(dict(call=ic, t=t, ccs=u))
                for j, cc in enumerate(u):
                    pairs.append(dict(tile=ti, j=j, cc=cc, stop=False))
        last = {}
        for j in range(p0, len(pairs)):
            last[pairs[j]["cc"]] = j
        for j in range(p0, len(pairs)):
            pairs[j]["stop"] = last[pairs[j]["cc"]] == j
        for cc in wm["chunks"]:
            assert cc in last, f"chunk {cc} has no edges in wave {v}"
        wm["tile0"] = t0
        wm["ntiles"] = len(tiles) - t0
        wm["pair0"] = p0
        wm["npairs"] = len(pairs) - p0
    NTILES = len(tiles)
    KMAX = max(len(tl["ccs"]) for tl in tiles)

    # --- per-core merged one-hot select columns (one per tile)
    dsel = np.full((c.C, 128, NTILES), 65535.0, np.float32)
    for ti, tl in enumerate(tiles):
        call = calls[tl["call"]]
        sl = slice(call["slot0"] + tl["t"] * 128, call["slot0"] + (tl["t"] + 1) * 128)
        schs = sch[:, sl]
        sdis = sdi[:, sl]
        for j, cc in enumerate(tl["ccs"]):
            m = schs == cc
            dsel[:, :, ti] = np.where(m, j * 128 + sdis, dsel[:, :, ti])

    cnt = np.bincount(batch, minlength=c.G).astype(np.float32)
    cntinv = (1.0 / np.maximum(cnt, 1.0)).astype(np.float32)

    iota_row = np.tile(np.arange(KMAX * 128, dtype=np.float32), (128, 1))
    identb = np.eye(128, dtype=np.float32).astype(NPBF)

    in_maps = []
    for ci in range(c.C):
        lo, hi = ci * c.NSH, (ci + 1) * c.NSH
        xT = np.zeros((c.IN, c.TROW), np.float32)
        xT[:, : c.NSH] = x[lo:hi].T
        dv = np.zeros((c.TROW,), np.float32)
        dv[: c.NSH] = dinv[lo:hi]
        dinv2d = dv.reshape(c.NCHK, 128).T.copy()

        oneh = np.zeros((c.TROW, 128), np.float32)
        oneh[np.arange(c.NSH), batch[lo:hi]] = 1.0

        bcols = np.stack([b1, b2, b3p], axis=1)
        b3rep = np.tile(b3p[None, :], (128, 1))
        bcrep = np.tile(bc[None, :], (128, 1))
        cinv = np.zeros((128, 1), np.float32)
        cinv[: c.G, 0] = cntinv

        in_maps.append(dict(
            xT=xT, dinv2d=dinv2d,
            gidx=_wrap_cols(gs[ci]),
            dsel=dsel[ci],
            oneh=oneh.astype(NPBF),
            W1d=W1, W2d=W2.astype(NPBF), W3d=W3p.astype(NPBF),
            bcols=bcols, b3rep=b3rep,
            Wcp=Wcp, bcrep=bcrep, cinv=cinv,
            iota=iota_row, identb=identb,
        ))

    meta = dict(calls=calls, waves=waves, tiles=tiles, pairs=pairs,
                TOT=TOT, NTILES=NTILES, KMAX=KMAX)
    return in_maps, meta


def build(cfg, meta):
    c = cfg
    HID, G, OUT = c.HID, c.G, c.OUT
    calls, waves, pairs = meta["calls"], meta["waves"], meta["pairs"]
    tiles = meta["tiles"]
    TOT, NTILES, KMAX = meta["TOT"], meta["NTILES"], meta["KMAX"]
    IDXC = max(wm["nslots"] // 16 for wm in waves)
    DSELC = max(wm["ntiles"] for wm in waves)
    MAXT = c.MAXCALL // 128     # 8 tiles per call

    nc = bacc.Bacc("TRN2", num_devices=c.C, dynamic_dma_scratch_size=65536,
                   num_swdge_queues=4)

    def ein(name, shape, dt=F32):
        return nc.dram_tensor(name, shape, dt, kind="ExternalInput")

    xT_d = ein("xT", [c.IN, c.TROW])
    dinv_d = ein("dinv2d", [128, c.NCHK])
    gidx_d = ein("gidx", [128, TOT // 16], I16)
    dsel_d = ein("dsel", [128, NTILES])
    oneh_d = ein("oneh", [c.TROW, 128], BF16)
    W1_d = ein("W1d", [c.IN, HID])
    W2_d = ein("W2d", [HID, HID], BF16)
    W3_d = ein("W3d", [HID, HID], BF16)
    bcols_d = ein("bcols", [HID, 3])
    b3rep_d = ein("b3rep", [128, HID])
    Wc_d = ein("Wcp", [HID, OUT])
    bcrep_d = ein("bcrep", [128, OUT])
    cinv_d = ein("cinv", [128, 1])
    iota_d = ein("iota", [128, KMAX * 128])
    identb_d = ein("identb", [128, 128], BF16)

    agin_d = nc.dram_tensor("agin", [c.TROW, 128], BF16, kind="Internal")
    agout_d = nc.dram_tensor(
        "agout", [c.C * c.TROW, 128], BF16, kind="Internal", addr_space="Shared")
    plin_d = nc.dram_tensor("plin", [128, HID], F32, kind="Internal")
    plout_d = nc.dram_tensor(
        "plout", [128, HID], F32, kind="Internal", addr_space="Shared")
    y_d = nc.dram_tensor("y", [G, OUT], F32, kind="ExternalOutput")

    rg = [list(range(c.C))]

    with tile.TileContext(nc) as tc:
        with (
            tc.tile_pool(name="res", bufs=1) as res,
            tc.tile_pool(name="stage", bufs=1) as stpool,
            tc.tile_pool(name="xt", bufs=3) as xtpool,
            tc.tile_pool(name="msg", bufs=28) as msgpool,
            tc.tile_pool(name="widx", bufs=2) as widxpool,
            tc.tile_pool(name="wdsel", bufs=2) as dselpool,
            tc.tile_pool(name="sel", bufs=6) as selpool,
            tc.tile_pool(name="fin", bufs=4) as finpool,
            tc.tile_pool(name="oh", bufs=2) as ohpool,
            tc.tile_pool(name="psA", bufs=2, space="PSUM") as psA,
            tc.tile_pool(name="psT", bufs=2, space="PSUM") as psT,
            tc.tile_pool(name="psW", bufs=2, space="PSUM") as psW,
        ):
            from concourse.masks import make_identity
            ident = res.tile([128, 128], F32)
            make_identity(nc, ident[:])
            identb_sb = res.tile([128, 128], BF16)
            nc.sync.dma_start(identb_sb[:], identb_d[:])
            iota_sb = res.tile([128, KMAX * 128], F32)
            nc.sync.dma_start(iota_sb[:], iota_d[:])
            dinv_sb = res.tile([128, c.NCHK], F32)
            nc.sync.dma_start(dinv_sb[:], dinv_d[:])
            W1_sb = res.tile([c.IN, HID], F32, name="w1")
            nc.sync.dma_start(W1_sb[:], W1_d[:])
            W2_sb = res.tile([HID, HID], BF16, name="w2")
            nc.sync.dma_start(W2_sb[:], W2_d[:])
            W3_sb = res.tile([HID, HID], BF16, name="w3")
            nc.sync.dma_start(W3_sb[:], W3_d[:])
            bcols_sb = res.tile([HID, 3], F32)
            nc.sync.dma_start(bcols_sb[:], bcols_d[:])
            b3rep_sb = res.tile([128, HID], F32)
            nc.sync.dma_start(b3rep_sb[:], b3rep_d[:])
            Wc_sb = res.tile([HID, OUT], F32)
            nc.sync.dma_start(Wc_sb[:], Wc_d[:])
            bcrep_sb = res.tile([128, OUT], F32)
            nc.sync.dma_start(bcrep_sb[:], bcrep_d[:])
            cinv_sb = res.tile([128, 1], F32)
            nc.sync.dma_start(cinv_sb[:], cinv_d[:])

            stage_sb = stpool.tile([128, c.NCHK, 128], BF16)
            hT_sb = stpool.tile([HID, c.TROW], BF16)
            h3_sb = stpool.tile([128, c.NCHK, HID], BF16)
            # message cols 64:128 of the table are never matmul'd but are
            # gathered; zero once so they are defined.
            nc.vector.memset(stage_sb[:, :, HID:], 0.0)

            agin_r = agin_d[:].rearrange("(k p) f -> p k f", p=128)
            oneh_r = oneh_d[:].rearrange("(k p) f -> p k f", p=128)

            nreg = nc.gpsimd.alloc_register("nidx")
            _regval = [None]

            def set_nreg(v):
                if _regval[0] != v:
                    nc.gpsimd.reg_mov(nreg, v)
                    _regval[0] = v

            BK = 8
            for l in range(3):
                # ---- transform: stage = dinv * (h @ W) ----
                for kk in range(cdiv(c.NCHK, BK)):
                    s = kk * BK
                    e = min(c.NCHK, s + BK)
                    if l == 0:
                        xt = xtpool.tile([c.IN, BK * 128], F32, tag="xt")
                        nc.sync.dma_start(
                            xt[:, : (e - s) * 128],
                            xT_d[:, s * 128:e * 128])
                    for k in range(s, e):
                        if l == 0:
                            lhsT = xt[:, (k - s) * 128:(k - s + 1) * 128]
                            Wl = W1_sb
                        else:
                            lhsT = hT_sb[:, k * 128:(k + 1) * 128]
                            Wl = W2_sb if l == 1 else W3_sb
                        ps = psA.tile([128, HID], F32, space="PSUM")
                        nc.tensor.matmul(ps[:], lhsT, Wl[:], start=True, stop=True)
                        nc.scalar.activation(
                            stage_sb[:, k, :HID], ps[:], AF.Copy,
                            scale=dinv_sb[:, k:k + 1])
                for kk in range(cdiv(c.NCHK, BK)):
                    s = kk * BK
                    e = min(c.NCHK, s + BK)
                    nc.sync.dma_start(agin_r[:, s:e, :], stage_sb[:, s:e, :])
                nc.gpsimd.collective_compute(
                    "AllGather", ALU.bypass,
                    replica_groups=rg, ins=[agin_d[:]], outs=[agout_d[:]])

                # ---- waves: gather + one-hot matmul aggregation ----
                for v, wm in enumerate(waves):
                    widx = widxpool.tile([128, IDXC], I16, tag="widx",
                                         name=f"widx_{l}_{v}")
                    col0 = wm["slot0"] // 16
                    ncol = wm["nslots"] // 16
                    nc.sync.dma_start(widx[:, :ncol], gidx_d[:, col0:col0 + ncol])
                    wdsel = dselpool.tile([128, DSELC], F32, tag="wdsel",
                                          name=f"wdsel_{l}_{v}")
                    nc.sync.dma_start(
                        wdsel[:, : wm["ntiles"]],
                        dsel_d[:, wm["tile0"]:wm["tile0"] + wm["ntiles"]])

                    # seed accumulators with self-loop term t' (all chunk
                    # accumulators of a wave share one PSUM bank tile)
                    ppw = psW.tile([128, c.W * HID], F32, space="PSUM",
                                   tag="pp", name=f"pp_{l}_{v}")
                    pp = {}
                    for i, cc in enumerate(wm["chunks"]):
                        pp[cc] = ppw[:, i * HID:(i + 1) * HID]
                        nc.tensor.matmul(
                            pp[cc], identb_sb[:], stage_sb[:, cc, :HID],
                            start=True, stop=False)

                    # gathers (one per bucket)
                    mtiles = {}
                    for ic in wm["calls"]:
                        call = calls[ic]
                        n = call["n"]
                        b = call["b"]
                        mt = msgpool.tile([128, MAXT, 128], BF16, tag="msg",
                                          name=f"msg_{l}_{ic}")
                        lo = (call["slot0"] - wm["slot0"]) // 16
                        set_nreg(n)
                        nc.gpsimd.dma_gather(
                            mt[:, : n // 128, :],
                            agout_d[b * c.SRCW:(b + 1) * c.SRCW, :],
                            widx[:, lo:lo + n // 16], n, nreg, 128,
                            queue_num=ic % 4, single_packet=False)
                        mtiles[ic] = mt

                    # one-hot matmul accumulation: one wide DVE build per
                    # tile, one accumulate matmul per (tile, chunk) pair
                    cur_ti, st = -1, None
                    for j in range(wm["pair0"], wm["pair0"] + wm["npairs"]):
                        pr = pairs[j]
                        ti = pr["tile"]
                        tl = tiles[ti]
                        if ti != cur_ti:
                            K = len(tl["ccs"])
                            st = selpool.tile([128, KMAX * 128], BF16,
                                              tag="sel", name=f"sel_{l}_{ti}")
                            nc.vector.tensor_scalar(
                                st[:, : K * 128], iota_sb[:, : K * 128],
                                wdsel[:, ti - wm["tile0"]:ti - wm["tile0"] + 1],
                                None, ALU.is_equal)
                            cur_ti = ti
                        nc.tensor.matmul(
                            pp[pr["cc"]],
                            st[:, pr["j"] * 128:(pr["j"] + 1) * 128],
                            mtiles[tl["call"]][:, tl["t"], :HID],
                            start=False, stop=pr["stop"])

                    # finalize chunks of this wave (post-scale on Scalar)
                    for cc in wm["chunks"]:
                        vv = finpool.tile([128, HID], F32, tag="v")
                        nc.scalar.activation(
                            vv[:], pp[cc], AF.Copy, scale=dinv_sb[:, cc:cc + 1])
                        if l < 2:
                            pt = psT.tile([HID, 128], F32, space="PSUM", tag="tp")
                            nc.tensor.transpose(pt[:], vv[:], ident[:])
                            nc.scalar.activation(
                                hT_sb[:, cc * 128:(cc + 1) * 128], pt[:],
                                AF.Relu, bias=bcols_sb[:, l:l + 1])
                        else:
                            vb = finpool.tile([128, HID], F32, tag="vb")
                            nc.vector.tensor_add(vb[:], vv[:], b3rep_sb[:])
                            nc.vector.tensor_relu(h3_sb[:, cc, :], vb[:])

            # ---- mean pool ----
            pq = psA.tile([128, HID], F32, space="PSUM", tag="pool", bufs=1)
            for kk in range(cdiv(c.NCHK, BK)):
                s = kk * BK
                e = min(c.NCHK, s + BK)
                oh = ohpool.tile([128, BK, 128], BF16, tag="oh")
                nc.sync.dma_start(oh[:, : e - s, :], oneh_r[:, s:e, :])
                for k in range(s, e):
                    nc.tensor.matmul(
                        pq[:], oh[:, k - s, :], h3_sb[:, k, :],
                        start=(k == 0), stop=(k == c.NCHK - 1))
            pl = res.tile([128, HID], F32)
            nc.vector.tensor_copy(pl[:], pq[:])
            nc.sync.dma_start(plin_d[:], pl[:])
            nc.gpsimd.collective_compute(
                "AllReduce", ALU.add,
                replica_groups=rg, ins=[plin_d[:]], outs=[plout_d[:]])
            plr = res.tile([128, HID], F32)
            nc.sync.dma_start(plr[:], plout_d[:])
            plm = res.tile([128, HID], F32)
            nc.vector.tensor_scalar_mul(plm[:], plr[:], cinv_sb[:])
            pst = psT.tile([HID, 128], F32, space="PSUM", tag="tp")
            nc.tensor.transpose(pst[:], plm[:], ident[:])
            plT = res.tile([HID, 128], F32)
            nc.vector.tensor_copy(plT[:], pst[:])
            psC = psT.tile([G, OUT], F32, space="PSUM", tag="pC", bufs=1)
            nc.tensor.matmul(psC[:], plT[:, :G], Wc_sb[:], start=True, stop=True)
            lg = res.tile([G, OUT], F32)
            nc.vector.tensor_add(lg[:], psC[:, :], bcrep_sb[:G, :])
            mx = res.tile([G, 1], F32)
            nc.vector.tensor_reduce(mx[:], lg[:], mybir.AxisListType.X, ALU.max)
            lgs = res.tile([G, OUT], F32)
            nc.vector.tensor_scalar_sub(lgs[:], lg[:], mx[:])
            ex = res.tile([G, OUT], F32)
            nc.scalar.activation(ex[:], lgs[:], AF.Exp)
            sm = res.tile([G, 1], F32)
            nc.vector.tensor_reduce(sm[:], ex[:], mybir.AxisListType.X, ALU.add)
            ls = res.tile([G, 1], F32)
            nc.scalar.activation(ls[:], sm[:], AF.Ln)
            yt = res.tile([G, OUT], F32)
            nc.vector.tensor_scalar_sub(yt[:], lgs[:], ls[:])
            nc.sync.dma_start(y_d[:], yt[:])

    return nc


def _finalize(nc):
    nc.compile()
    fix_multiwait(nc)


def run(inputs, cfg, profile_dir=None):
    from concourse.bass_utils import run_bass_kernel_spmd

    in_maps, meta = prep(inputs, cfg)
    nc = build(cfg, meta)
    _finalize(nc)
    if profile_dir is not None:
        from trn_agent_boot.trn_boot import _ntff_profile_via_ctypes
        hook = _ntff_profile_via_ctypes("/opt/axon/libaxon_pjrt.so")
        with hook(profile_dir, [0]):
            res = run_bass_kernel_spmd(nc, in_maps, core_ids=list(range(cfg.C)))
    else:
        res = run_bass_kernel_spmd(nc, in_maps, core_ids=list(range(cfg.C)))
    return res.results[0]["y"]


# ---------------------------------------------------------------------------
N_NODES, N_EDGES, IN_DIM, HID_DIM, N_GRAPHS, OUT_DIM = 100_000, 1_600_000, 128, 64, 128, 3


def kernel(**inputs):
    import os
    cfg = Cfg(N_NODES, N_EDGES, IN_DIM, HID_DIM, N_GRAPHS, OUT_DIM, W=7)
    out = run(inputs, cfg, profile_dir=os.environ.get("GNN_PROFILE_DIR"))
    return np.asarray(out, np.float32)
